# revision 24
# baseline (speedup 1.0000x reference)
"""Trainium2 Bass kernel for nn_LinkEncoding (gnn_message_passing).

Takes FULL inputs (as produced by reference.setup_inputs()), shards
data-parallel over nodes across 8 NeuronCores, runs a Bass/Tile kernel,
returns the FULL [N, OUT_CH] float32 output.

v3 design notes (vs v2, 3.42ms baseline):
  - All three LayerNorms' per-(slot,t) statistics for the iteration's
    in-flight groups {t: g, c: g-2, h: g-4} land in ONE [120,3,4,6]
    strip; a single batched chain (even/odd bn_stats recombine + magic
    rsqrt + 1 Newton step, 9 DVE ops on [120,12]) replaces 12 bn_aggr +
    3 separate 5-op chains per group.
  - LN_t normalize is 4 per-t tensor_scalar ops (x*ri+nmb in one pass,
    per-partition scalar operands).
  - ch1 bias b1 rides an extra ones-column in zcS (col 101) through the
    transpose into w1 row 101; the 4 per-j Gelus merge into 2 double-
    width Gelus (no per-j bias needed anymore).
  - ch2 bias b2 rides a warm ones-row in hcS (row 100) through the
    back-transpose via idf row 100; the DVE bias-add becomes an ACT
    copy.
  - TM16->acc copy moved DVE->ACT.
  - PSUM: Bx x4, Btp/TM16 1 bank, zcT/Bc/P2f 1 bank, Bh [100,2,512] 2
    banks = 8.
"""

import math
import os
import sys

for _p in ("/opt/trn_rl_repo", "/root/.axon_site/_ro/trn_rl_repo"):
    if os.path.isdir(_p) and _p not in sys.path:
        sys.path.append(_p)

import numpy as np
import ml_dtypes

BF16 = ml_dtypes.bfloat16

# Problem constants (hardcoded per harness contract).
N_NODES = 50000
E_EDGES = 800000
K = 30
HID = 100
TCH = 100
OUT_CH = 100
NCORES = 8

SLOT = 4 * K          # 120 slots (4 nodes) per sub-tile
GW = 16               # nodes per group (4 sub-tiles)
CH_G = 32             # groups per chunk (TM [101, 512] f32 = 1 bank)

MAGIC = 0x5F3759DF

_CACHE = {}
LAST_RESULT = None


def _bcast(ap, n):
    """Broadcast the last (size-1) free dim of `ap` to size n via stride 0."""
    import concourse.bass as bass
    a = list(ap.ap)
    assert a[-1][1] == 1, a
    a[-1] = [0, n]
    return bass.AP(tensor=ap.tensor, offset=ap.offset, ap=a)


def _bcast_mid(ap, n):
    """Insert a stride-0 dim of size n after the partition dim."""
    import concourse.bass as bass
    a = list(ap.ap)
    a.insert(1, [0, n])
    return bass.AP(tensor=ap.tensor, offset=ap.offset, ap=a)


def _build_nc(NG, lnt_identity):
    import concourse.bass as bass
    import concourse.tile as tile
    from concourse import bacc, mybir
    from contextlib import ExitStack

    f32 = mybir.dt.float32
    bf16 = mybir.dt.bfloat16
    i32 = mybir.dt.int32
    AF = mybir.ActivationFunctionType
    OP = mybir.AluOpType

    nc = bacc.Bacc(None, target_bir_lowering=False)

    xin = nc.dram_tensor("xin", [NG, 106, 4 * SLOT], bf16, kind="ExternalInput")
    wht = nc.dram_tensor("wht", [106, HID], bf16, kind="ExternalInput")
    t1 = nc.dram_tensor("t1", [SLOT, 64], bf16, kind="ExternalInput")
    t2 = nc.dram_tensor("t2", [64, SLOT], bf16, kind="ExternalInput")
    w1 = nc.dram_tensor("w1", [102, 4, HID], bf16, kind="ExternalInput")
    w2 = nc.dram_tensor("w2", [HID, 4, HID], bf16, kind="ExternalInput")
    owt = nc.dram_tensor("owt", [101, HID], f32, kind="ExternalInput")
    idb = nc.dram_tensor("idb", [SLOT, SLOT], bf16, kind="ExternalInput")
    idf = nc.dram_tensor("idf", [HID, HID], bf16, kind="ExternalInput")
    b2 = nc.dram_tensor("b2", [HID, 1], f32, kind="ExternalInput")
    ob = nc.dram_tensor("ob", [HID, 1], f32, kind="ExternalInput")
    t1b = nc.dram_tensor("t1b", [64, 1], f32, kind="ExternalInput")
    pk = nc.dram_tensor("pk", [SLOT, 4], f32, kind="ExternalInput")
    gtb = nc.dram_tensor("gtb", [SLOT, 2 * HID], f32, kind="ExternalInput")
    y2 = nc.dram_tensor("y2", [HID, NG * GW], f32, kind="ExternalOutput")

    with tile.TileContext(nc) as tc, ExitStack() as ctx:
        singles = ctx.enter_context(tc.tile_pool(name="singles", bufs=1))
        pgt = ctx.enter_context(tc.tile_pool(name="pgt", bufs=6))
        px = ctx.enter_context(tc.tile_pool(name="px", bufs=3))
        pgel = ctx.enter_context(tc.tile_pool(name="pgel", bufs=3))
        pzt = ctx.enter_context(tc.tile_pool(name="pzt", bufs=3))
        pgh = ctx.enter_context(tc.tile_pool(name="pgh", bufs=2))
        phc = ctx.enter_context(tc.tile_pool(name="phc", bufs=2))
        phts = ctx.enter_context(tc.tile_pool(name="phts", bufs=4))
        pstat = ctx.enter_context(tc.tile_pool(name="pstat", bufs=2))
        ptm = ctx.enter_context(tc.tile_pool(name="ptm", bufs=2))
        # PSUM pools: 3 (Bx) + 2 (btm, zcb) + 2 (bh) + 1 (btT) = 8 banks
        pbx = ctx.enter_context(tc.tile_pool(name="pbx", bufs=3, space="PSUM"))
        pps = ctx.enter_context(tc.tile_pool(name="pps", bufs=1, space="PSUM"))
        pbh = ctx.enter_context(tc.tile_pool(name="pbh", bufs=1, space="PSUM"))
        pbt = ctx.enter_context(tc.tile_pool(name="pbt", bufs=1, space="PSUM"))

        # --- constants -------------------------------------------------
        s_wht = singles.tile([106, HID], bf16)
        nc.sync.dma_start(s_wht[:], wht[:, :])
        s_t1 = singles.tile([SLOT, 64], bf16)
        nc.sync.dma_start(s_t1[:], t1[:, :])
        s_t2 = singles.tile([64, SLOT], bf16)
        nc.sync.dma_start(s_t2[:], t2[:, :])
        s_w1 = singles.tile([102, 4, HID], bf16)
        nc.sync.dma_start(s_w1[:], w1[:, :, :])
        s_w2 = singles.tile([HID, 4, HID], bf16)
        nc.sync.dma_start(s_w2[:], w2[:, :, :])
        s_owt = singles.tile([101, HID], f32)
        nc.sync.dma_start(s_owt[:], owt[:, :])
        s_idb = singles.tile([SLOT, SLOT], bf16)
        nc.sync.dma_start(s_idb[:], idb[:, :])
        s_idf = singles.tile([HID, HID], bf16)
        nc.sync.dma_start(s_idf[:], idf[:, :])
        s_b2 = singles.tile([HID, 1], f32)
        nc.sync.dma_start(s_b2[:], b2[:, :])
        s_ob = singles.tile([HID, 1], f32)
        nc.sync.dma_start(s_ob[:], ob[:, :])
        s_t1b = singles.tile([64, 1], f32)
        nc.sync.dma_start(s_t1b[:], t1b[:, :])
        s_pk = singles.tile([SLOT, 4], f32)
        nc.sync.dma_start(s_pk[:], pk[:, :])
        s_gtb = singles.tile([SLOT, 2 * HID], f32)
        nc.sync.dma_start(s_gtb[:], gtb[:, :])

        nchunks = (NG + CH_G - 1) // CH_G
        PF = 3                     # DMA prefetch depth (groups)
        gts = {}                   # g -> GTs tile
        state = {}                 # g -> per-group tiles live across stages
        accs = {}                  # ci -> SBUF TM accumulator

        def issue_dma(g):
            GTs = pgt.tile([106, 4 * SLOT], bf16, tag="gt", name=f"gts{g}")
            nc.sync.dma_start(GTs[:], xin[g, :, :])
            gts[g] = GTs

        def P1(g):
            GTs = gts.pop(g)
            Bx = pbx.tile([SLOT, 4, HID], f32, tag="bx", name=f"bx{g}")
            for t in range(4):
                nc.tensor.matmul(Bx[:, t, :],
                                 GTs[:, t * SLOT:(t + 1) * SLOT],
                                 s_wht[:], start=(t == 0), stop=True,
                                 skip_group_check=True)
            state[g] = {"Bx": Bx}

        def P2b(g):
            Bx = state[g]["Bx"]
            gel = state[g].pop("gel")
            # tok2 (merged): h_token = x + t2.T @ gel   (accumulate)
            nc.tensor.matmul(Bx[:, :, :], s_t2[:], gel[:, :, :],
                             start=False, stop=True, skip_group_check=True)

        def strip(i):
            """Per-iteration stats strip: slot 0=LN_t(i), 1=LN_c(i-2),
            2=LN_h(i-5)."""
            st = pstat.tile([SLOT, 3, 4, 6], f32, tag="st", name=f"st{i}")
            if not (5 <= i < NG):
                nc.vector.memset(st[:], 0.0)
            return st

        def stats_into(st, sl, src_fn):
            for t in range(4):
                nc.vector.bn_stats(st[:, sl, t, :], src_fn(t))

        def chain(i, st):
            """Batched even/odd recombine + magic rsqrt for all 3 LN slots
            (DVE, end of iteration; consumed next iteration).

            st fields per (ln,t): [50, m_e, cv_e, 50, m_o, cv_o].
            ve = cv_e + cv_o + 25*(m_e-m_o)^2 + 1e-3 = 100*(var + 1e-5);
            ri = 10/sqrt(ve) = 1/sqrt(var+eps) (x10 folded into the last
            Newton step); mm = m_e + m_o = 2*mean; nmb = -mean*ri_t.
            """
            v = nc.vector
            mk = lambda tg: pstat.tile([SLOT, 3, 4], f32, tag=tg,
                                       name=f"{tg}{i}")
            d = mk("chd")
            v.tensor_tensor(d[:], st[:, :, :, 1], st[:, :, :, 4],
                            op=OP.subtract)
            s = mk("chs")
            v.scalar_tensor_tensor(s[:], st[:, :, :, 2], 1e-3,
                                   st[:, :, :, 5], op0=OP.add, op1=OP.add)
            d2 = mk("chd2")
            v.tensor_tensor(d2[:], d[:], d[:], op=OP.mult)
            ve = mk("chve")
            v.scalar_tensor_tensor(ve[:], d2[:], 25.0, s[:], op0=OP.mult,
                                   op1=OP.add)
            y0 = mk("chy0")
            v.tensor_scalar(y0[:].bitcast(i32), ve[:].bitcast(i32), -0.5,
                            float(MAGIC), op0=OP.mult, op1=OP.add)
            t1n = mk("cht1")
            v.tensor_tensor(t1n[:], y0[:], y0[:], op=OP.mult)
            t2n = mk("cht2")
            v.scalar_tensor_tensor(t2n[:], t1n[:], -5.0, ve[:], op0=OP.mult,
                                   op1=OP.mult)
            ri = mk("chri")
            v.scalar_tensor_tensor(ri[:], t2n[:], 15.0, y0[:], op0=OP.add,
                                   op1=OP.mult)
            mm = mk("chmm")
            v.tensor_tensor(mm[:], st[:, :, :, 1], st[:, :, :, 4], op=OP.add)
            nmb = pstat.tile([SLOT, 4], f32, tag="chnm", name=f"chnm{i}")
            v.scalar_tensor_tensor(nmb[:], mm[:, 0, :], -0.5, ri[:, 0, :],
                                   op0=OP.mult, op1=OP.mult)
            return ri, mm, nmb

        def HtS(g):
            """h_token -> bf16 SBUF; frees Bx's PSUM bank (ACT)."""
            Bx = state[g].pop("Bx")
            htS = phts.tile([SLOT, 4, HID], bf16, tag="hts", name=f"hts{g}")
            nc.scalar.activation(htS[:], Bx[:, :, :], AF.Copy)
            state[g]["htS"] = htS

        def V1n(g, riP, nmbP):
            """LN_t normalize: xS = (x - m)*ri, bf16 SBUF (DVE)."""
            Bx = state[g]["Bx"]
            xS = px.tile([SLOT, 4, HID], bf16, tag="xs", name=f"xs{g}")
            nc.vector.tensor_tensor(xS[:, :, :], Bx[:, :, :],
                                    _bcast(riP[:, 0, :, None], 100),
                                    op=OP.mult)
            nc.vector.tensor_tensor(xS[:, :, :], xS[:, :, :],
                                    _bcast(nmbP[:, :, None], 100), op=OP.add)
            if not lnt_identity:
                gt_ = _bcast_mid(s_gtb[:, 0:100], 4)
                bt_ = _bcast_mid(s_gtb[:, 100:200], 4)
                tmp = px.tile([SLOT, 4, HID], f32, tag="lngt")
                nc.vector.scalar_tensor_tensor(
                    tmp[:], xS[:, :, :], 1.0, gt_, op0=OP.mult, op1=OP.mult)
                nc.vector.tensor_tensor(xS[:, :, :], tmp[:], bt_, op=OP.add)
            state[g]["xS"] = xS

        def V2n(g, riP, mmP):
            """LN_c normalize into zcS from bf16 h_token (DVE 4x mode)."""
            htS = state[g]["htS"]
            zcS = px.tile([SLOT, 4, 102], bf16, tag="zc", name=f"zc{g}")
            nc.vector.memset(zcS[:, :, 101], 1.0)   # ones col -> b1 row
            nc.vector.scalar_tensor_tensor(zcS[:, :, 100], mmP[:, 1, :],
                                           -0.5, riP[:, 1, :], op0=OP.mult,
                                           op1=OP.mult)
            for t in range(4):
                nc.vector.tensor_scalar(zcS[:, t, 0:100], htS[:, t, :],
                                        riP[:, 1, t:t + 1], None,
                                        op0=OP.mult)
            state[g]["zcS"] = zcS

        def V3bR(g, riP, mmP):
            """LN_h: hhS aug col = -mean; R = pk (x) ri_h (DVE)."""
            hhS = state[g]["hhS"]
            nc.vector.tensor_scalar(hhS[:, :, 100], mmP[:, 2, :], -0.5,
                                    None, op0=OP.mult)
            R = pstat.tile([SLOT, 4, 4], bf16, tag="R", name=f"R{g}")
            nc.vector.tensor_tensor(R[:], _bcast_mid(s_pk[:, :], 4),
                                    _bcast(riP[:, 2, :, None], 4),
                                    op=OP.mult)
            state[g]["R"] = R

        def P2a(g):
            xS = state[g].pop("xS")
            Btp = pps.tile([64, 4, HID], f32, tag="btm", bufs=1,
                           name=f"btp{g}")
            nc.tensor.matmul(Btp[:, :, :], s_t1[:], xS[:, :, :],
                             start=True, stop=True, skip_group_check=True)
            state[g]["Btp"] = Btp

        def P2a2(g):
            Btp = state[g].pop("Btp")
            gel = pgel.tile([64, 4, HID], bf16, tag="gel", name=f"gel{g}")
            nc.scalar.activation(gel[:], Btp[:, :, :], AF.Gelu,
                                 bias=s_t1b[:, 0:1], scale=1.0)
            state[g]["gel"] = gel

        def P3a(g):
            zcS = state[g].pop("zcS")
            # transpose: zcT [102, (4,120)] bf16 PSUM (rows 100=-m*ric, 101=1)
            zcT = pps.tile([102, 4, SLOT], bf16, tag="zcb", bufs=1,
                           name=f"zcT{g}")
            for t in range(4):
                nc.tensor.matmul(zcT[:, t, :], zcS[:, t, 0:102], s_idb[:],
                                 is_transpose=True, start=(t == 0),
                                 stop=True, skip_group_check=True)
            zs = pzt.tile([102, 4, SLOT], bf16, tag="zs", name=f"zs{g}")
            nc.scalar.activation(zs[:], zcT[:], AF.Copy)
            state[g]["zs"] = zs

        def P3b(g, half):
            """ch1 pair (j=2*half, 2*half+1) + merged gelu."""
            zs = state[g]["zs"]
            Bh = pbh.tile([HID, 2, 512], f32, tag="bh", name=f"bh{g}_{half}")
            for jj in range(2):
                j = 2 * half + jj
                nc.tensor.matmul(Bh[:, jj, 0:480], s_w1[:, j, :],
                                 zs[:, :, :], start=True, stop=True)
            gh = state[g].get("gh")
            if gh is None:
                gh = pgh.tile([HID, 4, 4 * SLOT], bf16, tag="gh",
                              name=f"gh{g}")
                state[g]["gh"] = gh
            nc.scalar.activation(gh[:, 2 * half:2 * half + 2, :],
                                 Bh[:, :, 0:480], AF.Gelu)

        def P3c(g):
            """ch2 (+b2 via ACT bias) + back-transpose into fresh btT."""
            gh = state[g].pop("gh")
            state[g].pop("zs")
            Bc = pps.tile([HID, 4 * SLOT], f32, tag="zcb", bufs=1,
                          name=f"bc{g}")
            for j in range(4):
                nc.tensor.matmul(Bc[:], s_w2[:, j, :], gh[:, j, :],
                                 start=(j == 0), stop=(j == 3))
            hcS = phc.tile([HID, 4 * SLOT], bf16, tag="hc", name=f"hc{g}")
            nc.scalar.activation(hcS[:], Bc[:], AF.Identity,
                                 bias=s_b2[:, 0:1], scale=1.0)
            btT = pbt.tile([SLOT, 4, HID], bf16, tag="btt", name=f"btt{g}")
            for t in range(4):
                nc.tensor.matmul(btT[:, t, :],
                                 hcS[:, t * SLOT:(t + 1) * SLOT],
                                 s_idf[:], is_transpose=True,
                                 start=(t == 0), stop=True,
                                 skip_group_check=True)
            state[g]["btT"] = btT

        def V3a(g):
            """h_channel = h_token + ch2^T (DVE bf16 add)."""
            htS = state[g].pop("htS")
            btT = state[g].pop("btT")
            hhS = px.tile([SLOT, 4, 102], bf16, tag="hh", name=f"hh{g}")
            nc.vector.tensor_tensor(hhS[:, :, 0:100], htS[:, :, :],
                                    btT[:, :, :], op=OP.add)
            state[g]["hhS"] = hhS

        def P4(g):
            st_ = state.pop(g)
            hhS, R = st_["hhS"], st_["R"]
            ci, gi = g // CH_G, g % CH_G
            if gi == 0:
                accs[ci] = ptm.tile([101, GW * CH_G], f32, tag="acc",
                                    name=f"acc{ci}")
            acc = accs[ci]
            TM16 = pps.tile([101, 16], f32, tag="btm", bufs=1,
                            name=f"tm{g}")
            for t in range(4):
                nc.tensor.matmul(TM16[:, 4 * t:4 * t + 4],
                                 hhS[:, t, 0:101], R[:, t, :],
                                 start=(t == 0), stop=True,
                                 skip_group_check=True)
            nc.vector.tensor_copy(acc[:, 16 * gi:16 * gi + 16], TM16[:])
            if gi == CH_G - 1 or g == NG - 1:
                finale(ci)

        def finale(ci):
            acc = accs.pop(ci)
            g0 = ci * CH_G
            gn = min(CH_G, NG - g0)
            nn = GW * gn
            P2f = pps.tile([HID, GW * CH_G], f32, tag="zcb", bufs=1,
                           name=f"p2f{ci}")
            nc.tensor.matmul(P2f[:, :nn], s_owt[:], acc[:, :nn],
                             start=True, stop=True)
            pj = ptm.tile([HID, GW * CH_G], f32, tag="pj", name=f"pj{ci}")
            nc.scalar.activation(pj[:, :nn], P2f[:, :nn], AF.Identity,
                                 bias=s_ob[:, 0:1], scale=1.0)
            nc.sync.dma_start(y2[:, g0 * GW:g0 * GW + nn], pj[:, :nn])

        # --- software-pipelined emission -------------------------------
        # chain(i) covers {LN_t: i, LN_c: i-2, LN_h: i-5}; its outputs are
        # consumed the NEXT iteration (riP/mmP/nmbP) so no engine waits on
        # the in-iteration DVE chain.
        riP = mmP = nmbP = None
        for g in range(min(PF, NG)):
            issue_dma(g)
        for i in range(NG + 6):
            if 0 <= i - 2 < NG:
                P2b(i - 2)
            if i < NG:
                P1(i)
                if i + PF < NG:
                    issue_dma(i + PF)
            if 0 <= i - 1 < NG:
                V1n(i - 1, riP, nmbP)
            if 0 <= i - 6 < NG:
                V3bR(i - 6, riP, mmP)
                P4(i - 6)
            if 0 <= i - 1 < NG:
                P2a(i - 1)
                P2a2(i - 1)
            if 0 <= i - 2 < NG:
                HtS(i - 2)
            if 0 <= i - 4 < NG:
                P3b(i - 4, 0)
            st = strip(i)
            if i < NG:
                Bx_ = state[i]["Bx"]
                stats_into(st, 0, lambda t: Bx_[:, t, :])
            if 0 <= i - 4 < NG:
                P3b(i - 4, 1)
            if 0 <= i - 3 < NG:
                V2n(i - 3, riP, mmP)
                P3a(i - 3)
            if 0 <= i - 2 < NG:
                ht_ = state[i - 2]["htS"]
                stats_into(st, 1, lambda t: ht_[:, t, :])
            if 0 <= i - 4 < NG:
                P3c(i - 4)
                V3a(i - 4)
            if 0 <= i - 5 < NG:
                hh_ = state[i - 5]["hhS"]
                stats_into(st, 2, lambda t: hh_[:, t, 0:100])
            riP, mmP, nmbP = chain(i, st)
    nc.compile()
    return nc


def _host_prepare(inputs):
    """Build per-core device input maps from the full problem inputs."""
    ea = np.asarray(inputs["edge_attr"], dtype=np.float32)
    et = np.asarray(inputs["edge_time"], dtype=np.float32)
    nb = np.asarray(inputs["node_batch"]).astype(np.int64)
    N = int(np.asarray(inputs["num_nodes"]))
    E = nb.shape[0]

    head_w = np.asarray(inputs["head_w"], dtype=np.float64)
    head_b = np.asarray(inputs["head_b"], dtype=np.float64)
    ln_t_g = np.asarray(inputs["ln_t_g"], dtype=np.float64)
    ln_t_b = np.asarray(inputs["ln_t_b"], dtype=np.float64)
    tok1_w = np.asarray(inputs["tok1_w"], dtype=np.float64)
    tok1_b = np.asarray(inputs["tok1_b"], dtype=np.float64)
    tok2_w = np.asarray(inputs["tok2_w"], dtype=np.float64)
    tok2_b = np.asarray(inputs["tok2_b"], dtype=np.float64)
    ln_c_g = np.asarray(inputs["ln_c_g"], dtype=np.float64)
    ln_c_b = np.asarray(inputs["ln_c_b"], dtype=np.float64)
    ch1_w = np.asarray(inputs["ch1_w"], dtype=np.float64)
    ch1_b = np.asarray(inputs["ch1_b"], dtype=np.float64)
    ch2_w = np.asarray(inputs["ch2_w"], dtype=np.float64)
    ch2_b = np.asarray(inputs["ch2_b"], dtype=np.float64)
    ln_h_g = np.asarray(inputs["ln_h_g"], dtype=np.float64)
    ln_h_b = np.asarray(inputs["ln_h_b"], dtype=np.float64)
    out_w = np.asarray(inputs["out_w"], dtype=np.float64)
    out_b = np.asarray(inputs["out_b"], dtype=np.float64)

    NPC = (N + NCORES - 1) // NCORES          # nodes per core
    NPCP = ((NPC + GW - 1) // GW) * GW        # padded to group multiple
    NG = NPCP // GW

    # --- edge -> slot assignment (stable sort, first K per node) ---
    order = np.argsort(nb, kind="stable")
    snb = nb[order]
    pos = np.arange(E, dtype=np.int64) - np.searchsorted(snb, snb, side="left")
    keep = pos < K
    le = order[keep]                 # edge ids, slot-ordered
    lnode = snb[keep]
    lk = pos[keep]
    core = (lnode // NPC).astype(np.int64)
    nl = (lnode % NPC).astype(np.int64)

    # --- dense slot table [cores, NPCP, K, 106] bf16 ---
    dense = np.zeros((NCORES, NPCP, K, 106), dtype=BF16)
    t64 = et[le].astype(np.float64)
    t2 = t64 * t64
    tp = np.stack([t2, t2 ** 2, t2 ** 3, t2 ** 4, t2 ** 5], axis=1)
    dense[core, nl, lk, 0:5] = tp.astype(np.float32)
    dense[core, nl, lk, 5:105] = ea[le]
    dense[core, nl, lk, 105] = np.float32(1.0)

    # --- folded weights ---
    sqrt_d = math.sqrt(TCH)
    tw = 1.0 / sqrt_d ** np.linspace(0.0, sqrt_d, TCH)  # float64
    W_time = head_w[:, :TCH]
    W_attr = head_w[:, TCH:]
    C = []
    for m in range(6):
        coef = ((-1.0) ** m) / math.factorial(2 * m)
        C.append(coef * (W_time @ (tw ** (2 * m))))     # [HID]
    wht = np.zeros((106, HID), dtype=np.float32)
    for m in range(1, 6):
        wht[m - 1, :] = C[m]
    wht[5:105, :] = W_attr.T
    wht[105, :] = head_b + C[0]

    lnt_identity = bool(np.allclose(ln_t_g, 1.0) and np.allclose(ln_t_b, 0.0))

    t1m = np.zeros((SLOT, 64), dtype=np.float32)
    t2m = np.zeros((64, SLOT), dtype=np.float32)
    for b in range(4):
        t1m[30 * b:30 * b + 30, 16 * b:16 * b + 15] = tok1_w.T
        t2m[16 * b:16 * b + 15, 30 * b:30 * b + 30] = tok2_w.T
    t1bv = np.zeros((64, 1), dtype=np.float32)
    for b in range(4):
        t1bv[16 * b:16 * b + 15, 0] = tok1_b
    # tok2_b dropped: constant per-slot shift is invariant under LN_c /
    # LN_h (which are the only consumers of h_token / h_channel).

    Wg1 = ch1_w * ln_c_g[None, :]
    b1p = ch1_b + ch1_w @ ln_c_b
    w1m = np.zeros((102, 4, HID), dtype=np.float32)
    for j in range(4):
        blk = Wg1[HID * j:HID * (j + 1), :].T          # [100c, 100h]
        w1m[0:100, j, :] = blk
        w1m[100, j, :] = blk.sum(axis=0)               # w1 row-sums
        w1m[101, j, :] = b1p[HID * j:HID * (j + 1)]    # ch1 bias (ones col)
    w2m = np.stack([ch2_w[:, HID * j:HID * (j + 1)].T for j in range(4)],
                   axis=1)                              # [100h, 4, 100c]

    OWg = out_w * ln_h_g[None, :]
    owtm = np.zeros((101, HID), dtype=np.float32)
    owtm[0:100, :] = OWg.T
    owtm[100, :] = OWg.sum(axis=1)
    obm = (out_b + out_w @ ln_h_b)[:, None]

    pkm = np.zeros((SLOT, 4), dtype=np.float32)
    for b in range(4):
        pkm[30 * b:30 * b + 30, b] = 1.0 / K

    gtbm = np.zeros((SLOT, 2 * HID), dtype=np.float32)
    gtbm[:, :HID] = ln_t_g[None, :]
    gtbm[:, HID:] = ln_t_b[None, :]

    base = {
        "wht": wht.astype(BF16),
        "t1": t1m.astype(BF16),
        "t2": t2m.astype(BF16),
        "w1": w1m.astype(BF16),
        "w2": w2m.astype(BF16),
        "owt": owtm.astype(np.float32),
        "idb": np.eye(SLOT, dtype=np.float32).astype(BF16),
        "idf": np.eye(HID, dtype=np.float32).astype(BF16),
        "b2": ch2_b[:, None].astype(np.float32),
        "ob": obm.astype(np.float32),
        "t1b": t1bv,
        "pk": pkm,
        "gtb": gtbm,
    }

    in_maps = []
    for c in range(NCORES):
        d = dense[c].reshape(NG, 4, 4, K, 106)       # [g, t, u, k, c]
        # pre-transposed: [g, feature, t, u*k] so the head matmul's lhsT
        # (GT) comes straight from DMA with no PE transpose
        d = np.ascontiguousarray(d.transpose(0, 4, 1, 2, 3))  # [g, c, t, u, k]
        d = d.reshape(NG, 106, 4 * SLOT)
        m = dict(base)
        m["xin"] = d
        in_maps.append(m)
    return in_maps, NG, NPC, NPCP, lnt_identity, N


def kernel(**inputs):
    global LAST_RESULT
    from concourse.bass_utils import run_bass_kernel_spmd

    in_maps, NG, NPC, NPCP, lnt_identity, N = _host_prepare(inputs)

    key = (NG, lnt_identity)
    if key not in _CACHE:
        _CACHE[key] = _build_nc(NG, lnt_identity)
    nc = _CACHE[key]

    res = run_bass_kernel_spmd(nc, in_maps, core_ids=list(range(NCORES)))
    LAST_RESULT = res

    parts = []
    remaining = N
    for c in range(NCORES):
        take = min(NPC, remaining)
        parts.append(res.results[c]["y2"].T[:take])
        remaining -= take
    out = np.ascontiguousarray(np.concatenate(parts, axis=0)).astype(np.float32)
    return out


# revision 25
# speedup vs baseline: 1.0146x; 1.0146x over previous
"""Trainium2 Bass kernel for nn_LinkEncoding (gnn_message_passing).

Takes FULL inputs (as produced by reference.setup_inputs()), shards
data-parallel over nodes across 8 NeuronCores, runs a Bass/Tile kernel,
returns the FULL [N, OUT_CH] float32 output.

v3 design notes (vs v2, 3.42ms baseline):
  - All three LayerNorms' per-(slot,t) statistics for the iteration's
    in-flight groups {t: g, c: g-2, h: g-4} land in ONE [120,3,4,6]
    strip; a single batched chain (even/odd bn_stats recombine + magic
    rsqrt + 1 Newton step, 9 DVE ops on [120,12]) replaces 12 bn_aggr +
    3 separate 5-op chains per group.
  - LN_t normalize is 4 per-t tensor_scalar ops (x*ri+nmb in one pass,
    per-partition scalar operands).
  - ch1 bias b1 rides an extra ones-column in zcS (col 101) through the
    transpose into w1 row 101; the 4 per-j Gelus merge into 2 double-
    width Gelus (no per-j bias needed anymore).
  - ch2 bias b2 rides a warm ones-row in hcS (row 100) through the
    back-transpose via idf row 100; the DVE bias-add becomes an ACT
    copy.
  - TM16->acc copy moved DVE->ACT.
  - PSUM: Bx x4, Btp/TM16 1 bank, zcT/Bc/P2f 1 bank, Bh [100,2,512] 2
    banks = 8.
"""

import math
import os
import sys

for _p in ("/opt/trn_rl_repo", "/root/.axon_site/_ro/trn_rl_repo"):
    if os.path.isdir(_p) and _p not in sys.path:
        sys.path.append(_p)

import numpy as np
import ml_dtypes

BF16 = ml_dtypes.bfloat16

# Problem constants (hardcoded per harness contract).
N_NODES = 50000
E_EDGES = 800000
K = 30
HID = 100
TCH = 100
OUT_CH = 100
NCORES = 8

SLOT = 4 * K          # 120 slots (4 nodes) per sub-tile
GW = 16               # nodes per group (4 sub-tiles)
CH_G = 32             # groups per chunk (TM [101, 512] f32 = 1 bank)

MAGIC = 0x5F3759DF

_CACHE = {}
LAST_RESULT = None


def _bcast(ap, n):
    """Broadcast the last (size-1) free dim of `ap` to size n via stride 0."""
    import concourse.bass as bass
    a = list(ap.ap)
    assert a[-1][1] == 1, a
    a[-1] = [0, n]
    return bass.AP(tensor=ap.tensor, offset=ap.offset, ap=a)


def _bcast_mid(ap, n):
    """Insert a stride-0 dim of size n after the partition dim."""
    import concourse.bass as bass
    a = list(ap.ap)
    a.insert(1, [0, n])
    return bass.AP(tensor=ap.tensor, offset=ap.offset, ap=a)


def _build_nc(NG, lnt_identity):
    import concourse.bass as bass
    import concourse.tile as tile
    from concourse import bacc, mybir
    from contextlib import ExitStack

    f32 = mybir.dt.float32
    bf16 = mybir.dt.bfloat16
    i32 = mybir.dt.int32
    AF = mybir.ActivationFunctionType
    OP = mybir.AluOpType

    nc = bacc.Bacc(None, target_bir_lowering=False)

    xin = nc.dram_tensor("xin", [NG, 106, 4 * SLOT], bf16, kind="ExternalInput")
    wht = nc.dram_tensor("wht", [106, HID], bf16, kind="ExternalInput")
    t1 = nc.dram_tensor("t1", [SLOT, 64], bf16, kind="ExternalInput")
    t2 = nc.dram_tensor("t2", [64, SLOT], bf16, kind="ExternalInput")
    w1 = nc.dram_tensor("w1", [102, 4, HID], bf16, kind="ExternalInput")
    w2 = nc.dram_tensor("w2", [HID, 4, HID], bf16, kind="ExternalInput")
    owt = nc.dram_tensor("owt", [101, HID], f32, kind="ExternalInput")
    idb = nc.dram_tensor("idb", [SLOT, SLOT], bf16, kind="ExternalInput")
    idf = nc.dram_tensor("idf", [HID, HID], bf16, kind="ExternalInput")
    b2 = nc.dram_tensor("b2", [HID, 1], f32, kind="ExternalInput")
    ob = nc.dram_tensor("ob", [HID, 1], f32, kind="ExternalInput")
    t1b = nc.dram_tensor("t1b", [64, 1], f32, kind="ExternalInput")
    pk = nc.dram_tensor("pk", [SLOT, 4], f32, kind="ExternalInput")
    gtb = nc.dram_tensor("gtb", [SLOT, 2 * HID], f32, kind="ExternalInput")
    y2 = nc.dram_tensor("y2", [HID, NG * GW], f32, kind="ExternalOutput")

    with tile.TileContext(nc) as tc, ExitStack() as ctx:
        singles = ctx.enter_context(tc.tile_pool(name="singles", bufs=1))
        pgt = ctx.enter_context(tc.tile_pool(name="pgt", bufs=6))
        px = ctx.enter_context(tc.tile_pool(name="px", bufs=3))
        pgel = ctx.enter_context(tc.tile_pool(name="pgel", bufs=3))
        pzt = ctx.enter_context(tc.tile_pool(name="pzt", bufs=3))
        pgh = ctx.enter_context(tc.tile_pool(name="pgh", bufs=2))
        phc = ctx.enter_context(tc.tile_pool(name="phc", bufs=2))
        phts = ctx.enter_context(tc.tile_pool(name="phts", bufs=4))
        pstat = ctx.enter_context(tc.tile_pool(name="pstat", bufs=2))
        ptm = ctx.enter_context(tc.tile_pool(name="ptm", bufs=2))
        # PSUM pools: 3 (Bx) + 2 (btm, zcb) + 2 (bh) + 1 (btT) = 8 banks
        pbx = ctx.enter_context(tc.tile_pool(name="pbx", bufs=3, space="PSUM"))
        pps = ctx.enter_context(tc.tile_pool(name="pps", bufs=1, space="PSUM"))
        pbh = ctx.enter_context(tc.tile_pool(name="pbh", bufs=1, space="PSUM"))
        pbt = ctx.enter_context(tc.tile_pool(name="pbt", bufs=1, space="PSUM"))

        # --- constants -------------------------------------------------
        s_wht = singles.tile([106, HID], bf16)
        nc.sync.dma_start(s_wht[:], wht[:, :])
        s_t1 = singles.tile([SLOT, 64], bf16)
        nc.sync.dma_start(s_t1[:], t1[:, :])
        s_t2 = singles.tile([64, SLOT], bf16)
        nc.sync.dma_start(s_t2[:], t2[:, :])
        s_w1 = singles.tile([102, 4, HID], bf16)
        nc.sync.dma_start(s_w1[:], w1[:, :, :])
        s_w2 = singles.tile([HID, 4, HID], bf16)
        nc.sync.dma_start(s_w2[:], w2[:, :, :])
        s_owt = singles.tile([101, HID], f32)
        nc.sync.dma_start(s_owt[:], owt[:, :])
        s_idb = singles.tile([SLOT, SLOT], bf16)
        nc.sync.dma_start(s_idb[:], idb[:, :])
        s_idf = singles.tile([HID, HID], bf16)
        nc.sync.dma_start(s_idf[:], idf[:, :])
        s_b2 = singles.tile([HID, 1], f32)
        nc.sync.dma_start(s_b2[:], b2[:, :])
        s_ob = singles.tile([HID, 1], f32)
        nc.sync.dma_start(s_ob[:], ob[:, :])
        s_t1b = singles.tile([64, 1], f32)
        nc.sync.dma_start(s_t1b[:], t1b[:, :])
        s_pk = singles.tile([SLOT, 4], f32)
        nc.sync.dma_start(s_pk[:], pk[:, :])
        s_gtb = singles.tile([SLOT, 2 * HID], f32)
        nc.sync.dma_start(s_gtb[:], gtb[:, :])

        nchunks = (NG + CH_G - 1) // CH_G
        PF = 3                     # DMA prefetch depth (groups)
        gts = {}                   # g -> GTs tile
        state = {}                 # g -> per-group tiles live across stages
        accs = {}                  # ci -> SBUF TM accumulator

        def issue_dma(g):
            GTs = pgt.tile([106, 4 * SLOT], bf16, tag="gt", name=f"gts{g}")
            nc.sync.dma_start(GTs[:], xin[g, :, :])
            gts[g] = GTs

        def P1(g):
            GTs = gts.pop(g)
            Bx = pbx.tile([SLOT, 4, HID], f32, tag="bx", name=f"bx{g}")
            for t in range(4):
                nc.tensor.matmul(Bx[:, t, :],
                                 GTs[:, t * SLOT:(t + 1) * SLOT],
                                 s_wht[:], start=(t == 0), stop=True,
                                 skip_group_check=True)
            state[g] = {"Bx": Bx}

        def P2b(g):
            Bx = state[g]["Bx"]
            gel = state[g].pop("gel")
            # tok2 (merged): h_token = x + t2.T @ gel   (accumulate)
            nc.tensor.matmul(Bx[:, :, :], s_t2[:], gel[:, :, :],
                             start=False, stop=True, skip_group_check=True)

        def strip(i):
            """Per-iteration stats strip: slot 0=LN_t(i), 1=LN_c(i-2),
            2=LN_h(i-5)."""
            st = pstat.tile([SLOT, 3, 4, 6], f32, tag="st", name=f"st{i}")
            if not (5 <= i < NG):
                nc.vector.memset(st[:], 0.0)
            return st

        def stats_into(st, sl, src_fn):
            for t in range(4):
                nc.vector.bn_stats(st[:, sl, t, :], src_fn(t))

        def chain(i, st):
            """Batched even/odd recombine + magic rsqrt for all 3 LN slots
            (DVE, end of iteration; consumed next iteration).

            st fields per (ln,t): [50, m_e, cv_e, 50, m_o, cv_o].
            ve = cv_e + cv_o + 25*(m_e-m_o)^2 + 1e-3 = 100*(var + 1e-5);
            ri = 10/sqrt(ve) = 1/sqrt(var+eps) (x10 folded into the last
            Newton step); mm = m_e + m_o = 2*mean; nmb = -mean*ri_t.
            """
            v = nc.vector
            mk = lambda tg: pstat.tile([SLOT, 3, 4], f32, tag=tg,
                                       name=f"{tg}{i}")
            d = mk("chd")
            v.tensor_tensor(d[:], st[:, :, :, 1], st[:, :, :, 4],
                            op=OP.subtract)
            s = mk("chs")
            v.scalar_tensor_tensor(s[:], st[:, :, :, 2], 1e-3,
                                   st[:, :, :, 5], op0=OP.add, op1=OP.add)
            d2 = mk("chd2")
            v.tensor_tensor(d2[:], d[:], d[:], op=OP.mult)
            ve = mk("chve")
            v.scalar_tensor_tensor(ve[:], d2[:], 25.0, s[:], op0=OP.mult,
                                   op1=OP.add)
            y0 = mk("chy0")
            v.tensor_scalar(y0[:].bitcast(i32), ve[:].bitcast(i32), -0.5,
                            float(MAGIC), op0=OP.mult, op1=OP.add)
            t1n = mk("cht1")
            v.tensor_tensor(t1n[:], y0[:], y0[:], op=OP.mult)
            t2n = mk("cht2")
            v.scalar_tensor_tensor(t2n[:], t1n[:], -5.0, ve[:], op0=OP.mult,
                                   op1=OP.mult)
            ri = mk("chri")
            v.scalar_tensor_tensor(ri[:], t2n[:], 15.0, y0[:], op0=OP.add,
                                   op1=OP.mult)
            mm = mk("chmm")
            v.tensor_tensor(mm[:], st[:, :, :, 1], st[:, :, :, 4], op=OP.add)
            nmb = pstat.tile([SLOT, 4], f32, tag="chnm", name=f"chnm{i}")
            v.scalar_tensor_tensor(nmb[:], mm[:, 0, :], -0.5, ri[:, 0, :],
                                   op0=OP.mult, op1=OP.mult)
            return ri, mm, nmb

        def HtS(g):
            """h_token -> bf16 SBUF; frees Bx's PSUM bank (ACT)."""
            Bx = state[g].pop("Bx")
            htS = phts.tile([SLOT, 4, HID], bf16, tag="hts", name=f"hts{g}")
            nc.scalar.activation(htS[:], Bx[:, :, :], AF.Copy)
            state[g]["htS"] = htS

        def V1n(g, riP, nmbP):
            """LN_t normalize: xS = (x - m)*ri, bf16 SBUF (DVE)."""
            Bx = state[g]["Bx"]
            xS = px.tile([SLOT, 4, HID], bf16, tag="xs", name=f"xs{g}")
            nc.vector.tensor_tensor(xS[:, :, :], Bx[:, :, :],
                                    _bcast(riP[:, 0, :, None], 100),
                                    op=OP.mult)
            nc.vector.tensor_tensor(xS[:, :, :], xS[:, :, :],
                                    _bcast(nmbP[:, :, None], 100), op=OP.add)
            if not lnt_identity:
                gt_ = _bcast_mid(s_gtb[:, 0:100], 4)
                bt_ = _bcast_mid(s_gtb[:, 100:200], 4)
                tmp = px.tile([SLOT, 4, HID], f32, tag="lngt")
                nc.vector.scalar_tensor_tensor(
                    tmp[:], xS[:, :, :], 1.0, gt_, op0=OP.mult, op1=OP.mult)
                nc.vector.tensor_tensor(xS[:, :, :], tmp[:], bt_, op=OP.add)
            state[g]["xS"] = xS

        def V2n(g, riP, mmP):
            """LN_c normalize into zcS from bf16 h_token (DVE 4x mode)."""
            htS = state[g]["htS"]
            zcS = px.tile([SLOT, 4, 102], bf16, tag="zc", name=f"zc{g}")
            nc.vector.memset(zcS[:, :, 101], 1.0)   # ones col -> b1 row
            nc.vector.scalar_tensor_tensor(zcS[:, :, 100], mmP[:, 1, :],
                                           -0.5, riP[:, 1, :], op0=OP.mult,
                                           op1=OP.mult)
            for t in range(4):
                nc.vector.tensor_scalar(zcS[:, t, 0:100], htS[:, t, :],
                                        riP[:, 1, t:t + 1], None,
                                        op0=OP.mult)
            state[g]["zcS"] = zcS

        def V3bR(g, riP, mmP):
            """LN_h: hhS aug col = -mean; R = pk (x) ri_h (DVE)."""
            hhS = state[g]["hhS"]
            nc.vector.tensor_scalar(hhS[:, :, 100], mmP[:, 2, :], -0.5,
                                    None, op0=OP.mult)
            R = pstat.tile([SLOT, 4, 4], bf16, tag="R", name=f"R{g}")
            nc.vector.tensor_tensor(R[:], _bcast_mid(s_pk[:, :], 4),
                                    _bcast(riP[:, 2, :, None], 4),
                                    op=OP.mult)
            state[g]["R"] = R

        def P2a(g):
            xS = state[g].pop("xS")
            Btp = pps.tile([64, 4, HID], f32, tag="btm", bufs=1,
                           name=f"btp{g}")
            nc.tensor.matmul(Btp[:, :, :], s_t1[:], xS[:, :, :],
                             start=True, stop=True, skip_group_check=True)
            state[g]["Btp"] = Btp

        def P2a2(g):
            Btp = state[g].pop("Btp")
            gel = pgel.tile([64, 4, HID], bf16, tag="gel", name=f"gel{g}")
            nc.scalar.activation(gel[:], Btp[:, :, :], AF.Gelu,
                                 bias=s_t1b[:, 0:1], scale=1.0)
            state[g]["gel"] = gel

        def P3a(g):
            zcS = state[g].pop("zcS")
            # transpose: zcT [102, (4,120)] bf16 PSUM (rows 100=-m*ric, 101=1)
            zcT = pps.tile([102, 4, SLOT], bf16, tag="zcb", bufs=1,
                           name=f"zcT{g}")
            for t in range(4):
                nc.tensor.matmul(zcT[:, t, :], zcS[:, t, 0:102], s_idb[:],
                                 is_transpose=True, start=(t == 0),
                                 stop=True, skip_group_check=True)
            zs = pzt.tile([102, 4, SLOT], bf16, tag="zs", name=f"zs{g}")
            nc.scalar.activation(zs[:], zcT[:], AF.Copy)
            state[g]["zs"] = zs

        def P3b(g, half):
            """ch1 pair (j=2*half, 2*half+1) + merged gelu."""
            zs = state[g]["zs"]
            Bh = pbh.tile([HID, 2, 512], f32, tag="bh", name=f"bh{g}_{half}")
            for jj in range(2):
                j = 2 * half + jj
                nc.tensor.matmul(Bh[:, jj, 0:480], s_w1[:, j, :],
                                 zs[:, :, :], start=True, stop=True)
            gh = state[g].get("gh")
            if gh is None:
                gh = pgh.tile([HID, 4, 4 * SLOT], bf16, tag="gh",
                              name=f"gh{g}")
                state[g]["gh"] = gh
            nc.scalar.activation(gh[:, 2 * half:2 * half + 2, :],
                                 Bh[:, :, 0:480], AF.Gelu)

        def P3c(g):
            """ch2 (+b2 via ACT bias) + back-transpose into fresh btT."""
            gh = state[g].pop("gh")
            state[g].pop("zs")
            Bc = pbt.tile([HID, 4 * SLOT], f32, tag="btt", bufs=1,
                          name=f"bc{g}")
            for j in range(4):
                nc.tensor.matmul(Bc[:], s_w2[:, j, :], gh[:, j, :],
                                 start=(j == 0), stop=(j == 3))
            hcS = phc.tile([HID, 4 * SLOT], bf16, tag="hc", name=f"hc{g}")
            nc.scalar.activation(hcS[:], Bc[:], AF.Identity,
                                 bias=s_b2[:, 0:1], scale=1.0)
            btT = pbt.tile([SLOT, 4, HID], bf16, tag="btt", name=f"btt{g}")
            for t in range(4):
                nc.tensor.matmul(btT[:, t, :],
                                 hcS[:, t * SLOT:(t + 1) * SLOT],
                                 s_idf[:], is_transpose=True,
                                 start=(t == 0), stop=True,
                                 skip_group_check=True)
            state[g]["btT"] = btT

        def V3a(g):
            """h_channel = h_token + ch2^T (DVE bf16 add)."""
            htS = state[g].pop("htS")
            btT = state[g].pop("btT")
            hhS = px.tile([SLOT, 4, 102], bf16, tag="hh", name=f"hh{g}")
            nc.vector.tensor_tensor(hhS[:, :, 0:100], htS[:, :, :],
                                    btT[:, :, :], op=OP.add)
            state[g]["hhS"] = hhS

        def P4(g):
            st_ = state.pop(g)
            hhS, R = st_["hhS"], st_["R"]
            ci, gi = g // CH_G, g % CH_G
            if gi == 0:
                accs[ci] = ptm.tile([101, GW * CH_G], f32, tag="acc",
                                    name=f"acc{ci}")
            acc = accs[ci]
            TM16 = pps.tile([101, 16], f32, tag="btm", bufs=1,
                            name=f"tm{g}")
            for t in range(4):
                nc.tensor.matmul(TM16[:, 4 * t:4 * t + 4],
                                 hhS[:, t, 0:101], R[:, t, :],
                                 start=(t == 0), stop=True,
                                 skip_group_check=True)
            nc.vector.tensor_copy(acc[:, 16 * gi:16 * gi + 16], TM16[:])
            if gi == CH_G - 1 or g == NG - 1:
                finale(ci)

        def finale(ci):
            acc = accs.pop(ci)
            g0 = ci * CH_G
            gn = min(CH_G, NG - g0)
            nn = GW * gn
            P2f = pps.tile([HID, GW * CH_G], f32, tag="zcb", bufs=1,
                           name=f"p2f{ci}")
            nc.tensor.matmul(P2f[:, :nn], s_owt[:], acc[:, :nn],
                             start=True, stop=True)
            pj = ptm.tile([HID, GW * CH_G], f32, tag="pj", name=f"pj{ci}")
            nc.scalar.activation(pj[:, :nn], P2f[:, :nn], AF.Identity,
                                 bias=s_ob[:, 0:1], scale=1.0)
            nc.sync.dma_start(y2[:, g0 * GW:g0 * GW + nn], pj[:, :nn])

        # --- software-pipelined emission -------------------------------
        # chain(i) covers {LN_t: i, LN_c: i-2, LN_h: i-5}; its outputs are
        # consumed the NEXT iteration (riP/mmP/nmbP) so no engine waits on
        # the in-iteration DVE chain.
        riP = mmP = nmbP = None
        for g in range(min(PF, NG)):
            issue_dma(g)
        for i in range(NG + 6):
            if 0 <= i - 2 < NG:
                P2b(i - 2)
            if i < NG:
                P1(i)
                if i + PF < NG:
                    issue_dma(i + PF)
            if 0 <= i - 1 < NG:
                V1n(i - 1, riP, nmbP)
            if 0 <= i - 6 < NG:
                V3bR(i - 6, riP, mmP)
                P4(i - 6)
            if 0 <= i - 1 < NG:
                P2a(i - 1)
                P2a2(i - 1)
            if 0 <= i - 2 < NG:
                HtS(i - 2)
            if 0 <= i - 4 < NG:
                P3b(i - 4, 0)
            st = strip(i)
            if i < NG:
                Bx_ = state[i]["Bx"]
                stats_into(st, 0, lambda t: Bx_[:, t, :])
            if 0 <= i - 4 < NG:
                P3b(i - 4, 1)
            if 0 <= i - 3 < NG:
                V2n(i - 3, riP, mmP)
                P3a(i - 3)
            if 0 <= i - 2 < NG:
                ht_ = state[i - 2]["htS"]
                stats_into(st, 1, lambda t: ht_[:, t, :])
            if 0 <= i - 4 < NG:
                P3c(i - 4)
                V3a(i - 4)
            if 0 <= i - 5 < NG:
                hh_ = state[i - 5]["hhS"]
                stats_into(st, 2, lambda t: hh_[:, t, 0:100])
            riP, mmP, nmbP = chain(i, st)
    nc.compile()
    return nc


def _host_prepare(inputs):
    """Build per-core device input maps from the full problem inputs."""
    ea = np.asarray(inputs["edge_attr"], dtype=np.float32)
    et = np.asarray(inputs["edge_time"], dtype=np.float32)
    nb = np.asarray(inputs["node_batch"]).astype(np.int64)
    N = int(np.asarray(inputs["num_nodes"]))
    E = nb.shape[0]

    head_w = np.asarray(inputs["head_w"], dtype=np.float64)
    head_b = np.asarray(inputs["head_b"], dtype=np.float64)
    ln_t_g = np.asarray(inputs["ln_t_g"], dtype=np.float64)
    ln_t_b = np.asarray(inputs["ln_t_b"], dtype=np.float64)
    tok1_w = np.asarray(inputs["tok1_w"], dtype=np.float64)
    tok1_b = np.asarray(inputs["tok1_b"], dtype=np.float64)
    tok2_w = np.asarray(inputs["tok2_w"], dtype=np.float64)
    tok2_b = np.asarray(inputs["tok2_b"], dtype=np.float64)
    ln_c_g = np.asarray(inputs["ln_c_g"], dtype=np.float64)
    ln_c_b = np.asarray(inputs["ln_c_b"], dtype=np.float64)
    ch1_w = np.asarray(inputs["ch1_w"], dtype=np.float64)
    ch1_b = np.asarray(inputs["ch1_b"], dtype=np.float64)
    ch2_w = np.asarray(inputs["ch2_w"], dtype=np.float64)
    ch2_b = np.asarray(inputs["ch2_b"], dtype=np.float64)
    ln_h_g = np.asarray(inputs["ln_h_g"], dtype=np.float64)
    ln_h_b = np.asarray(inputs["ln_h_b"], dtype=np.float64)
    out_w = np.asarray(inputs["out_w"], dtype=np.float64)
    out_b = np.asarray(inputs["out_b"], dtype=np.float64)

    NPC = (N + NCORES - 1) // NCORES          # nodes per core
    NPCP = ((NPC + GW - 1) // GW) * GW        # padded to group multiple
    NG = NPCP // GW

    # --- edge -> slot assignment (stable sort, first K per node) ---
    order = np.argsort(nb, kind="stable")
    snb = nb[order]
    pos = np.arange(E, dtype=np.int64) - np.searchsorted(snb, snb, side="left")
    keep = pos < K
    le = order[keep]                 # edge ids, slot-ordered
    lnode = snb[keep]
    lk = pos[keep]
    core = (lnode // NPC).astype(np.int64)
    nl = (lnode % NPC).astype(np.int64)

    # --- dense slot table [cores, NPCP, K, 106] bf16 ---
    dense = np.zeros((NCORES, NPCP, K, 106), dtype=BF16)
    t64 = et[le].astype(np.float64)
    t2 = t64 * t64
    tp = np.stack([t2, t2 ** 2, t2 ** 3, t2 ** 4, t2 ** 5], axis=1)
    dense[core, nl, lk, 0:5] = tp.astype(np.float32)
    dense[core, nl, lk, 5:105] = ea[le]
    dense[core, nl, lk, 105] = np.float32(1.0)

    # --- folded weights ---
    sqrt_d = math.sqrt(TCH)
    tw = 1.0 / sqrt_d ** np.linspace(0.0, sqrt_d, TCH)  # float64
    W_time = head_w[:, :TCH]
    W_attr = head_w[:, TCH:]
    C = []
    for m in range(6):
        coef = ((-1.0) ** m) / math.factorial(2 * m)
        C.append(coef * (W_time @ (tw ** (2 * m))))     # [HID]
    wht = np.zeros((106, HID), dtype=np.float32)
    for m in range(1, 6):
        wht[m - 1, :] = C[m]
    wht[5:105, :] = W_attr.T
    wht[105, :] = head_b + C[0]

    lnt_identity = bool(np.allclose(ln_t_g, 1.0) and np.allclose(ln_t_b, 0.0))

    t1m = np.zeros((SLOT, 64), dtype=np.float32)
    t2m = np.zeros((64, SLOT), dtype=np.float32)
    for b in range(4):
        t1m[30 * b:30 * b + 30, 16 * b:16 * b + 15] = tok1_w.T
        t2m[16 * b:16 * b + 15, 30 * b:30 * b + 30] = tok2_w.T
    t1bv = np.zeros((64, 1), dtype=np.float32)
    for b in range(4):
        t1bv[16 * b:16 * b + 15, 0] = tok1_b
    # tok2_b dropped: constant per-slot shift is invariant under LN_c /
    # LN_h (which are the only consumers of h_token / h_channel).

    Wg1 = ch1_w * ln_c_g[None, :]
    b1p = ch1_b + ch1_w @ ln_c_b
    w1m = np.zeros((102, 4, HID), dtype=np.float32)
    for j in range(4):
        blk = Wg1[HID * j:HID * (j + 1), :].T          # [100c, 100h]
        w1m[0:100, j, :] = blk
        w1m[100, j, :] = blk.sum(axis=0)               # w1 row-sums
        w1m[101, j, :] = b1p[HID * j:HID * (j + 1)]    # ch1 bias (ones col)
    w2m = np.stack([ch2_w[:, HID * j:HID * (j + 1)].T for j in range(4)],
                   axis=1)                              # [100h, 4, 100c]

    OWg = out_w * ln_h_g[None, :]
    owtm = np.zeros((101, HID), dtype=np.float32)
    owtm[0:100, :] = OWg.T
    owtm[100, :] = OWg.sum(axis=1)
    obm = (out_b + out_w @ ln_h_b)[:, None]

    pkm = np.zeros((SLOT, 4), dtype=np.float32)
    for b in range(4):
        pkm[30 * b:30 * b + 30, b] = 1.0 / K

    gtbm = np.zeros((SLOT, 2 * HID), dtype=np.float32)
    gtbm[:, :HID] = ln_t_g[None, :]
    gtbm[:, HID:] = ln_t_b[None, :]

    base = {
        "wht": wht.astype(BF16),
        "t1": t1m.astype(BF16),
        "t2": t2m.astype(BF16),
        "w1": w1m.astype(BF16),
        "w2": w2m.astype(BF16),
        "owt": owtm.astype(np.float32),
        "idb": np.eye(SLOT, dtype=np.float32).astype(BF16),
        "idf": np.eye(HID, dtype=np.float32).astype(BF16),
        "b2": ch2_b[:, None].astype(np.float32),
        "ob": obm.astype(np.float32),
        "t1b": t1bv,
        "pk": pkm,
        "gtb": gtbm,
    }

    in_maps = []
    for c in range(NCORES):
        d = dense[c].reshape(NG, 4, 4, K, 106)       # [g, t, u, k, c]
        # pre-transposed: [g, feature, t, u*k] so the head matmul's lhsT
        # (GT) comes straight from DMA with no PE transpose
        d = np.ascontiguousarray(d.transpose(0, 4, 1, 2, 3))  # [g, c, t, u, k]
        d = d.reshape(NG, 106, 4 * SLOT)
        m = dict(base)
        m["xin"] = d
        in_maps.append(m)
    return in_maps, NG, NPC, NPCP, lnt_identity, N


def kernel(**inputs):
    global LAST_RESULT
    from concourse.bass_utils import run_bass_kernel_spmd

    in_maps, NG, NPC, NPCP, lnt_identity, N = _host_prepare(inputs)

    key = (NG, lnt_identity)
    if key not in _CACHE:
        _CACHE[key] = _build_nc(NG, lnt_identity)
    nc = _CACHE[key]

    res = run_bass_kernel_spmd(nc, in_maps, core_ids=list(range(NCORES)))
    LAST_RESULT = res

    parts = []
    remaining = N
    for c in range(NCORES):
        take = min(NPC, remaining)
        parts.append(res.results[c]["y2"].T[:take])
        remaining -= take
    out = np.ascontiguousarray(np.concatenate(parts, axis=0)).astype(np.float32)
    return out


# revision 39
# speedup vs baseline: 1.0154x; 1.0009x over previous
"""Trainium2 Bass kernel for nn_LinkEncoding (gnn_message_passing).

Takes FULL inputs (as produced by reference.setup_inputs()), shards
data-parallel over nodes across 8 NeuronCores, runs a Bass/Tile kernel,
returns the FULL [N, OUT_CH] float32 output.

v3 design notes (vs v2, 3.42ms baseline):
  - All three LayerNorms' per-(slot,t) statistics for the iteration's
    in-flight groups {t: g, c: g-2, h: g-4} land in ONE [120,3,4,6]
    strip; a single batched chain (even/odd bn_stats recombine + magic
    rsqrt + 1 Newton step, 9 DVE ops on [120,12]) replaces 12 bn_aggr +
    3 separate 5-op chains per group.
  - LN_t normalize is 4 per-t tensor_scalar ops (x*ri+nmb in one pass,
    per-partition scalar operands).
  - ch1 bias b1 rides an extra ones-column in zcS (col 101) through the
    transpose into w1 row 101; the 4 per-j Gelus merge into 2 double-
    width Gelus (no per-j bias needed anymore).
  - ch2 bias b2 rides a warm ones-row in hcS (row 100) through the
    back-transpose via idf row 100; the DVE bias-add becomes an ACT
    copy.
  - TM16->acc copy moved DVE->ACT.
  - PSUM: Bx x4, Btp/TM16 1 bank, zcT/Bc/P2f 1 bank, Bh [100,2,512] 2
    banks = 8.
"""

import math
import os
import sys

for _p in ("/opt/trn_rl_repo", "/root/.axon_site/_ro/trn_rl_repo"):
    if os.path.isdir(_p) and _p not in sys.path:
        sys.path.append(_p)

import numpy as np
import ml_dtypes

BF16 = ml_dtypes.bfloat16
FP8 = ml_dtypes.float8_e4m3

# Problem constants (hardcoded per harness contract).
N_NODES = 50000
E_EDGES = 800000
K = 30
HID = 100
TCH = 100
OUT_CH = 100
NCORES = 8

SLOT = 4 * K          # 120 slots (4 nodes) per sub-tile
GW = 16               # nodes per group (4 sub-tiles)
CH_G = 32             # groups per chunk (TM [101, 512] f32 = 1 bank)

MAGIC = 0x5F3759DF

_CACHE = {}
LAST_RESULT = None


def _bcast(ap, n):
    """Broadcast the last (size-1) free dim of `ap` to size n via stride 0."""
    import concourse.bass as bass
    a = list(ap.ap)
    assert a[-1][1] == 1, a
    a[-1] = [0, n]
    return bass.AP(tensor=ap.tensor, offset=ap.offset, ap=a)


def _bcast_mid(ap, n):
    """Insert a stride-0 dim of size n after the partition dim."""
    import concourse.bass as bass
    a = list(ap.ap)
    a.insert(1, [0, n])
    return bass.AP(tensor=ap.tensor, offset=ap.offset, ap=a)


def _build_nc(NG, lnt_identity):
    import concourse.bass as bass
    import concourse.tile as tile
    from concourse import bacc, mybir
    from contextlib import ExitStack

    f32 = mybir.dt.float32
    bf16 = mybir.dt.bfloat16
    i32 = mybir.dt.int32
    AF = mybir.ActivationFunctionType
    OP = mybir.AluOpType

    nc = bacc.Bacc(None, target_bir_lowering=False)

    xin = nc.dram_tensor("xin", [NG, 106, 4 * SLOT], bf16, kind="ExternalInput")
    wht = nc.dram_tensor("wht", [106, HID], bf16, kind="ExternalInput")
    t1 = nc.dram_tensor("t1", [SLOT, 64], bf16, kind="ExternalInput")
    t2 = nc.dram_tensor("t2", [64, SLOT], bf16, kind="ExternalInput")
    fp8 = mybir.dt.float8e4
    w1 = nc.dram_tensor("w1", [102, 4, HID], bf16, kind="ExternalInput")
    w2 = nc.dram_tensor("w2", [HID, 4, HID], bf16, kind="ExternalInput")
    owt = nc.dram_tensor("owt", [101, HID], f32, kind="ExternalInput")
    idb = nc.dram_tensor("idb", [SLOT, SLOT], bf16, kind="ExternalInput")
    idf = nc.dram_tensor("idf", [HID, HID], bf16, kind="ExternalInput")
    b2 = nc.dram_tensor("b2", [HID, 1], f32, kind="ExternalInput")
    ob = nc.dram_tensor("ob", [HID, 1], f32, kind="ExternalInput")
    t1b = nc.dram_tensor("t1b", [64, 1], f32, kind="ExternalInput")
    pk = nc.dram_tensor("pk", [SLOT, 4], f32, kind="ExternalInput")
    gtb = nc.dram_tensor("gtb", [SLOT, 2 * HID], f32, kind="ExternalInput")
    y2 = nc.dram_tensor("y2", [HID, NG * GW], f32, kind="ExternalOutput")

    with tile.TileContext(nc) as tc, ExitStack() as ctx:
        singles = ctx.enter_context(tc.tile_pool(name="singles", bufs=1))
        pgt = ctx.enter_context(tc.tile_pool(name="pgt", bufs=6))
        px = ctx.enter_context(tc.tile_pool(name="px", bufs=3))
        pgel = ctx.enter_context(tc.tile_pool(name="pgel", bufs=3))
        pzt = ctx.enter_context(tc.tile_pool(name="pzt", bufs=3))
        pgh = ctx.enter_context(tc.tile_pool(name="pgh", bufs=2))
        phc = ctx.enter_context(tc.tile_pool(name="phc", bufs=2))
        phts = ctx.enter_context(tc.tile_pool(name="phts", bufs=4))
        pstat = ctx.enter_context(tc.tile_pool(name="pstat", bufs=2))
        ptm = ctx.enter_context(tc.tile_pool(name="ptm", bufs=2))
        # PSUM pools: 3 (Bx) + 2 (btm, zcb) + 2 (bh) + 1 (btT) = 8 banks
        pbx = ctx.enter_context(tc.tile_pool(name="pbx", bufs=3, space="PSUM"))
        pps = ctx.enter_context(tc.tile_pool(name="pps", bufs=1, space="PSUM"))
        pbh = ctx.enter_context(tc.tile_pool(name="pbh", bufs=1, space="PSUM"))
        pbt = ctx.enter_context(tc.tile_pool(name="pbt", bufs=1, space="PSUM"))

        # --- constants -------------------------------------------------
        s_wht = singles.tile([106, HID], bf16)
        nc.sync.dma_start(s_wht[:], wht[:, :])
        s_t1 = singles.tile([SLOT, 64], bf16)
        nc.sync.dma_start(s_t1[:], t1[:, :])
        s_t2 = singles.tile([64, SLOT], bf16)
        nc.sync.dma_start(s_t2[:], t2[:, :])
        s_w1 = singles.tile([102, 4, HID], bf16)
        nc.sync.dma_start(s_w1[:], w1[:, :, :])
        s_w2 = singles.tile([HID, 4, HID], bf16)
        nc.sync.dma_start(s_w2[:], w2[:, :, :])
        s_owt = singles.tile([101, HID], f32)
        nc.sync.dma_start(s_owt[:], owt[:, :])
        s_idb = singles.tile([SLOT, SLOT], bf16)
        nc.sync.dma_start(s_idb[:], idb[:, :])
        s_idf = singles.tile([HID, HID], bf16)
        nc.sync.dma_start(s_idf[:], idf[:, :])
        s_b2 = singles.tile([HID, 1], f32)
        nc.sync.dma_start(s_b2[:], b2[:, :])
        s_ob = singles.tile([HID, 1], f32)
        nc.sync.dma_start(s_ob[:], ob[:, :])
        s_t1b = singles.tile([64, 1], f32)
        nc.sync.dma_start(s_t1b[:], t1b[:, :])
        s_pk = singles.tile([SLOT, 4], f32)
        nc.sync.dma_start(s_pk[:], pk[:, :])
        s_gtb = singles.tile([SLOT, 2 * HID], f32)
        nc.sync.dma_start(s_gtb[:], gtb[:, :])

        nchunks = (NG + CH_G - 1) // CH_G
        PF = 3                     # DMA prefetch depth (groups)
        gts = {}                   # g -> GTs tile
        state = {}                 # g -> per-group tiles live across stages
        accs = {}                  # ci -> SBUF TM accumulator

        def issue_dma(g):
            GTs = pgt.tile([106, 4 * SLOT], bf16, tag="gt", name=f"gts{g}")
            nc.sync.dma_start(GTs[:], xin[g, :, :])
            gts[g] = GTs

        def P1(g):
            GTs = gts.pop(g)
            Bx = pbx.tile([SLOT, 4, HID], f32, tag="bx", name=f"bx{g}")
            for t in range(4):
                nc.tensor.matmul(Bx[:, t, :],
                                 GTs[:, t * SLOT:(t + 1) * SLOT],
                                 s_wht[:], start=(t == 0), stop=True,
                                 skip_group_check=True)
            state[g] = {"Bx": Bx}

        def P2b(g):
            Bx = state[g]["Bx"]
            gel = state[g].pop("gel")
            # tok2 (merged): h_token = x + t2.T @ gel   (accumulate)
            nc.tensor.matmul(Bx[:, :, :], s_t2[:], gel[:, :, :],
                             start=False, stop=True, skip_group_check=True)

        def strip(i):
            """Per-iteration stats strip: slot 0=LN_t(i), 1=LN_c(i-2),
            2=LN_h(i-5)."""
            st = pstat.tile([SLOT, 3, 4, 6], f32, tag="st", name=f"st{i}")
            if not (5 <= i < NG):
                nc.vector.memset(st[:], 0.0)
            return st

        def stats_into(st, sl, src_fn):
            for t in range(4):
                nc.vector.bn_stats(st[:, sl, t, :], src_fn(t))

        def chain(i, st):
            """Batched even/odd recombine + magic rsqrt for all 3 LN slots
            (DVE, end of iteration; consumed next iteration).

            st fields per (ln,t): [50, m_e, cv_e, 50, m_o, cv_o].
            ve = cv_e + cv_o + 25*(m_e-m_o)^2 + 1e-3 = 100*(var + 1e-5);
            ri = 10/sqrt(ve) = 1/sqrt(var+eps) (x10 folded into the last
            Newton step); mm = m_e + m_o = 2*mean; nmb = -mean*ri_t.
            """
            v = nc.vector
            mk = lambda tg: pstat.tile([SLOT, 3, 4], f32, tag=tg,
                                       name=f"{tg}{i}")
            d = mk("chd")
            v.tensor_tensor(d[:], st[:, :, :, 1], st[:, :, :, 4],
                            op=OP.subtract)
            s = mk("chs")
            v.scalar_tensor_tensor(s[:], st[:, :, :, 2], 1e-3,
                                   st[:, :, :, 5], op0=OP.add, op1=OP.add)
            d2 = mk("chd2")
            v.tensor_tensor(d2[:], d[:], d[:], op=OP.mult)
            ve = mk("chve")
            v.scalar_tensor_tensor(ve[:], d2[:], 25.0, s[:], op0=OP.mult,
                                   op1=OP.add)
            y0 = mk("chy0")
            v.tensor_scalar(y0[:].bitcast(i32), ve[:].bitcast(i32), -0.5,
                            float(MAGIC), op0=OP.mult, op1=OP.add)
            t1n = mk("cht1")
            v.tensor_tensor(t1n[:], y0[:], y0[:], op=OP.mult)
            t2n = mk("cht2")
            v.scalar_tensor_tensor(t2n[:], t1n[:], -5.0, ve[:], op0=OP.mult,
                                   op1=OP.mult)
            ri = mk("chri")
            v.scalar_tensor_tensor(ri[:], t2n[:], 15.0, y0[:], op0=OP.add,
                                   op1=OP.mult)
            mm = mk("chmm")
            v.tensor_tensor(mm[:], st[:, :, :, 1], st[:, :, :, 4], op=OP.add)
            nmb = pstat.tile([SLOT, 4], f32, tag="chnm", name=f"chnm{i}")
            v.scalar_tensor_tensor(nmb[:], mm[:, 0, :], -0.5, ri[:, 0, :],
                                   op0=OP.mult, op1=OP.mult)
            return ri, mm, nmb

        def HtS(g):
            """h_token -> bf16 SBUF; frees Bx's PSUM bank (ACT)."""
            Bx = state[g].pop("Bx")
            htS = phts.tile([SLOT, 4, HID], bf16, tag="hts", name=f"hts{g}")
            nc.scalar.activation(htS[:], Bx[:, :, :], AF.Copy)
            state[g]["htS"] = htS

        def V1n(g, riP, nmbP):
            """LN_t normalize: xS = (x - m)*ri, bf16 SBUF (DVE)."""
            Bx = state[g]["Bx"]
            xS = px.tile([SLOT, 4, HID], bf16, tag="xs", name=f"xs{g}")
            nc.vector.tensor_tensor(xS[:, :, :], Bx[:, :, :],
                                    _bcast(riP[:, 0, :, None], 100),
                                    op=OP.mult)
            nc.vector.tensor_tensor(xS[:, :, :], xS[:, :, :],
                                    _bcast(nmbP[:, :, None], 100), op=OP.add)
            if not lnt_identity:
                gt_ = _bcast_mid(s_gtb[:, 0:100], 4)
                bt_ = _bcast_mid(s_gtb[:, 100:200], 4)
                tmp = px.tile([SLOT, 4, HID], f32, tag="lngt")
                nc.vector.scalar_tensor_tensor(
                    tmp[:], xS[:, :, :], 1.0, gt_, op0=OP.mult, op1=OP.mult)
                nc.vector.tensor_tensor(xS[:, :, :], tmp[:], bt_, op=OP.add)
            state[g]["xS"] = xS

        def V2n(g, riP, mmP):
            """LN_c normalize into zcS from bf16 h_token (DVE 4x mode).

            Col 100 = -m*ric, col 101 = 1 (b1 row), cols 102:128 pad
            (memset 1.0, matched by zero rows of w1)."""
            htS = state[g]["htS"]
            zcS = px.tile([SLOT, 4, 102], bf16, tag="zc", name=f"zc{g}")
            nc.vector.memset(zcS[:, :, 101], 1.0)
            nc.vector.scalar_tensor_tensor(zcS[:, :, 100], mmP[:, 1, :],
                                           -0.5, riP[:, 1, :], op0=OP.mult,
                                           op1=OP.mult)
            for t in range(4):
                nc.vector.tensor_scalar(zcS[:, t, 0:100], htS[:, t, :],
                                        riP[:, 1, t:t + 1], None,
                                        op0=OP.mult)
            state[g]["zcS"] = zcS

        def V3bR(g, riP, mmP):
            """LN_h: hhS aug col = -mean; R = pk (x) ri_h (DVE)."""
            hhS = state[g]["hhS"]
            nc.vector.tensor_scalar(hhS[:, :, 100], mmP[:, 2, :], -0.5,
                                    None, op0=OP.mult)
            R = pstat.tile([SLOT, 4, 4], bf16, tag="R", name=f"R{g}")
            nc.vector.tensor_tensor(R[:], _bcast_mid(s_pk[:, :], 4),
                                    _bcast(riP[:, 2, :, None], 4),
                                    op=OP.mult)
            state[g]["R"] = R

        def P2a(g):
            xS = state[g].pop("xS")
            Btp = pps.tile([64, 4, HID], f32, tag="btm", bufs=1,
                           name=f"btp{g}")
            nc.tensor.matmul(Btp[:, :, :], s_t1[:], xS[:, :, :],
                             start=True, stop=True, skip_group_check=True)
            state[g]["Btp"] = Btp

        def P2a2(g):
            Btp = state[g].pop("Btp")
            gel = pgel.tile([64, 4, HID], bf16, tag="gel", name=f"gel{g}")
            nc.scalar.activation(gel[:], Btp[:, :, :], AF.Gelu,
                                 bias=s_t1b[:, 0:1], scale=1.0)
            state[g]["gel"] = gel

        def P3a(g):
            zcS = state[g].pop("zcS")
            # transpose all 128 (padded) channels; the DoubleRow i=1 group
            # is a broadcast duplicate matched by zero rows in w1
            zcT = pps.tile([102, 4, SLOT], bf16, tag="zcb", bufs=1,
                           name=f"zcT{g}")
            for t in range(4):
                nc.tensor.matmul(zcT[:, t, :], zcS[:, t, 0:102], s_idb[:],
                                 is_transpose=True, start=(t == 0),
                                 stop=True, skip_group_check=True)
            zs = pzt.tile([102, 4, SLOT], bf16, tag="zs", name=f"zs{g}")
            nc.scalar.activation(zs[:], zcT[:], AF.Copy)
            state[g]["zs"] = zs

        def P3b(g, half):
            """ch1 pair (j=2*half, 2*half+1), fp8 DoubleRow + merged gelu."""
            zs = state[g]["zs"]
            Bh = pbh.tile([HID, 2, 512], f32, tag="bh", name=f"bh{g}_{half}")
            for jj in range(2):
                j = 2 * half + jj
                nc.tensor.matmul(Bh[:, jj, 0:480], s_w1[:, j, :],
                                 zs[:, :, :], start=True, stop=True)
            gh = state[g].get("gh")
            if gh is None:
                gh = pgh.tile([HID, 4, 4 * SLOT], bf16, tag="gh",
                              name=f"gh{g}")
                state[g]["gh"] = gh
            nc.scalar.activation(gh[:, 2 * half:2 * half + 2, :],
                                 Bh[:, :, 0:480], AF.Gelu)

        def P3c(g):
            """ch2 (fp8 DoubleRow over j-block pairs) + back-transpose."""
            gh = state[g].pop("gh")
            state[g].pop("zs")
            Bc = pbt.tile([HID, 4 * SLOT], f32, tag="btt", bufs=1,
                          name=f"bc{g}")
            for j in range(4):
                nc.tensor.matmul(Bc[:], s_w2[:, j, :], gh[:, j, :],
                                 start=(j == 0), stop=(j == 3))
            hcS = phc.tile([HID, 4 * SLOT], bf16, tag="hc", name=f"hc{g}")
            nc.scalar.activation(hcS[:], Bc[:], AF.Identity,
                                 bias=s_b2[:, 0:1], scale=1.0)
            btT = pbt.tile([SLOT, 4, HID], bf16, tag="btt", name=f"btt{g}")
            for t in range(4):
                nc.tensor.matmul(btT[:, t, :],
                                 hcS[:, t * SLOT:(t + 1) * SLOT],
                                 s_idf[:], is_transpose=True,
                                 start=(t == 0), stop=True,
                                 skip_group_check=True)
            state[g]["btT"] = btT

        def V3a(g):
            """h_channel = h_token + ch2^T (DVE bf16 add)."""
            htS = state[g].pop("htS")
            btT = state[g].pop("btT")
            hhS = px.tile([SLOT, 4, 102], bf16, tag="hh", name=f"hh{g}")
            nc.vector.tensor_tensor(hhS[:, :, 0:100], htS[:, :, :],
                                    btT[:, :, :], op=OP.add)
            state[g]["hhS"] = hhS

        def P4(g):
            st_ = state.pop(g)
            hhS, R = st_["hhS"], st_["R"]
            ci, gi = g // CH_G, g % CH_G
            if gi == 0:
                accs[ci] = ptm.tile([101, GW * CH_G], f32, tag="acc",
                                    name=f"acc{ci}")
            acc = accs[ci]
            TM16 = pps.tile([101, 16], f32, tag="btm", bufs=1,
                            name=f"tm{g}")
            for t in range(4):
                nc.tensor.matmul(TM16[:, 4 * t:4 * t + 4],
                                 hhS[:, t, 0:101], R[:, t, :],
                                 start=(t == 0), stop=True,
                                 skip_group_check=True)
            nc.vector.tensor_copy(acc[:, 16 * gi:16 * gi + 16], TM16[:])
            if gi == CH_G - 1 or g == NG - 1:
                finale(ci)

        def finale(ci):
            acc = accs.pop(ci)
            g0 = ci * CH_G
            gn = min(CH_G, NG - g0)
            nn = GW * gn
            P2f = pps.tile([HID, GW * CH_G], f32, tag="zcb", bufs=1,
                           name=f"p2f{ci}")
            nc.tensor.matmul(P2f[:, :nn], s_owt[:], acc[:, :nn],
                             start=True, stop=True)
            pj = ptm.tile([HID, GW * CH_G], f32, tag="pj", name=f"pj{ci}")
            nc.scalar.activation(pj[:, :nn], P2f[:, :nn], AF.Identity,
                                 bias=s_ob[:, 0:1], scale=1.0)
            nc.sync.dma_start(y2[:, g0 * GW:g0 * GW + nn], pj[:, :nn])

        # --- software-pipelined emission -------------------------------
        # chain(i) covers {LN_t: i, LN_c: i-2, LN_h: i-5}; its outputs are
        # consumed the NEXT iteration (riP/mmP/nmbP) so no engine waits on
        # the in-iteration DVE chain.
        riP = mmP = nmbP = None
        for g in range(min(PF, NG)):
            issue_dma(g)
        for i in range(NG + 6):
            if 0 <= i - 2 < NG:
                P2b(i - 2)
            if i < NG:
                P1(i)
                if i + PF < NG:
                    issue_dma(i + PF)
            if 0 <= i - 1 < NG:
                V1n(i - 1, riP, nmbP)
            if 0 <= i - 6 < NG:
                V3bR(i - 6, riP, mmP)
                P4(i - 6)
            if 0 <= i - 1 < NG:
                P2a(i - 1)
                P2a2(i - 1)
            if 0 <= i - 2 < NG:
                HtS(i - 2)
            if 0 <= i - 4 < NG:
                P3b(i - 4, 0)
            st = strip(i)
            if i < NG:
                Bx_ = state[i]["Bx"]
                stats_into(st, 0, lambda t: Bx_[:, t, :])
            if 0 <= i - 4 < NG:
                P3b(i - 4, 1)
            if 0 <= i - 3 < NG:
                V2n(i - 3, riP, mmP)
                P3a(i - 3)
            if 0 <= i - 2 < NG:
                ht_ = state[i - 2]["htS"]
                stats_into(st, 1, lambda t: ht_[:, t, :])
            if 0 <= i - 4 < NG:
                P3c(i - 4)
                V3a(i - 4)
            if 0 <= i - 5 < NG:
                hh_ = state[i - 5]["hhS"]
                stats_into(st, 2, lambda t: hh_[:, t, 0:100])
            riP, mmP, nmbP = chain(i, st)
    nc.compile()
    return nc


def _host_prepare(inputs):
    """Build per-core device input maps from the full problem inputs."""
    ea = np.asarray(inputs["edge_attr"], dtype=np.float32)
    et = np.asarray(inputs["edge_time"], dtype=np.float32)
    nb = np.asarray(inputs["node_batch"]).astype(np.int64)
    N = int(np.asarray(inputs["num_nodes"]))
    E = nb.shape[0]

    head_w = np.asarray(inputs["head_w"], dtype=np.float64)
    head_b = np.asarray(inputs["head_b"], dtype=np.float64)
    ln_t_g = np.asarray(inputs["ln_t_g"], dtype=np.float64)
    ln_t_b = np.asarray(inputs["ln_t_b"], dtype=np.float64)
    tok1_w = np.asarray(inputs["tok1_w"], dtype=np.float64)
    tok1_b = np.asarray(inputs["tok1_b"], dtype=np.float64)
    tok2_w = np.asarray(inputs["tok2_w"], dtype=np.float64)
    tok2_b = np.asarray(inputs["tok2_b"], dtype=np.float64)
    ln_c_g = np.asarray(inputs["ln_c_g"], dtype=np.float64)
    ln_c_b = np.asarray(inputs["ln_c_b"], dtype=np.float64)
    ch1_w = np.asarray(inputs["ch1_w"], dtype=np.float64)
    ch1_b = np.asarray(inputs["ch1_b"], dtype=np.float64)
    ch2_w = np.asarray(inputs["ch2_w"], dtype=np.float64)
    ch2_b = np.asarray(inputs["ch2_b"], dtype=np.float64)
    ln_h_g = np.asarray(inputs["ln_h_g"], dtype=np.float64)
    ln_h_b = np.asarray(inputs["ln_h_b"], dtype=np.float64)
    out_w = np.asarray(inputs["out_w"], dtype=np.float64)
    out_b = np.asarray(inputs["out_b"], dtype=np.float64)

    NPC = (N + NCORES - 1) // NCORES          # nodes per core
    NPCP = ((NPC + GW - 1) // GW) * GW        # padded to group multiple
    NG = NPCP // GW

    # --- edge -> slot assignment (stable sort, first K per node) ---
    order = np.argsort(nb, kind="stable")
    snb = nb[order]
    pos = np.arange(E, dtype=np.int64) - np.searchsorted(snb, snb, side="left")
    keep = pos < K
    le = order[keep]                 # edge ids, slot-ordered
    lnode = snb[keep]
    lk = pos[keep]
    core = (lnode // NPC).astype(np.int64)
    nl = (lnode % NPC).astype(np.int64)

    # --- dense slot table [cores, NPCP, K, 106] bf16 ---
    dense = np.zeros((NCORES, NPCP, K, 106), dtype=BF16)
    t64 = et[le].astype(np.float64)
    t2 = t64 * t64
    tp = np.stack([t2, t2 ** 2, t2 ** 3, t2 ** 4, t2 ** 5], axis=1)
    dense[core, nl, lk, 0:5] = tp.astype(np.float32)
    dense[core, nl, lk, 5:105] = ea[le]
    dense[core, nl, lk, 105] = np.float32(1.0)

    # --- folded weights ---
    sqrt_d = math.sqrt(TCH)
    tw = 1.0 / sqrt_d ** np.linspace(0.0, sqrt_d, TCH)  # float64
    W_time = head_w[:, :TCH]
    W_attr = head_w[:, TCH:]
    C = []
    for m in range(6):
        coef = ((-1.0) ** m) / math.factorial(2 * m)
        C.append(coef * (W_time @ (tw ** (2 * m))))     # [HID]
    wht = np.zeros((106, HID), dtype=np.float32)
    for m in range(1, 6):
        wht[m - 1, :] = C[m]
    wht[5:105, :] = W_attr.T
    wht[105, :] = head_b + C[0]

    lnt_identity = bool(np.allclose(ln_t_g, 1.0) and np.allclose(ln_t_b, 0.0))

    t1m = np.zeros((SLOT, 64), dtype=np.float32)
    t2m = np.zeros((64, SLOT), dtype=np.float32)
    for b in range(4):
        t1m[30 * b:30 * b + 30, 16 * b:16 * b + 15] = tok1_w.T
        t2m[16 * b:16 * b + 15, 30 * b:30 * b + 30] = tok2_w.T
    t1bv = np.zeros((64, 1), dtype=np.float32)
    for b in range(4):
        t1bv[16 * b:16 * b + 15, 0] = tok1_b
    # tok2_b dropped: constant per-slot shift is invariant under LN_c /
    # LN_h (which are the only consumers of h_token / h_channel).

    Wg1 = ch1_w * ln_c_g[None, :]
    b1p = ch1_b + ch1_w @ ln_c_b
    w1m = np.zeros((102, 4, HID), dtype=np.float32)
    for j in range(4):
        blk = Wg1[HID * j:HID * (j + 1), :].T          # [100c, 100h]
        w1m[0:100, j, :] = blk
        w1m[100, j, :] = blk.sum(axis=0)               # w1 row-sums
        w1m[101, j, :] = b1p[HID * j:HID * (j + 1)]    # ch1 bias (ones col)
    w2m = np.stack([ch2_w[:, HID * j:HID * (j + 1)].T for j in range(4)],
                   axis=1)                              # [100h, 4, 100c]

    OWg = out_w * ln_h_g[None, :]
    owtm = np.zeros((101, HID), dtype=np.float32)
    owtm[0:100, :] = OWg.T
    owtm[100, :] = OWg.sum(axis=1)
    obm = (out_b + out_w @ ln_h_b)[:, None]

    pkm = np.zeros((SLOT, 4), dtype=np.float32)
    for b in range(4):
        pkm[30 * b:30 * b + 30, b] = 1.0 / K

    gtbm = np.zeros((SLOT, 2 * HID), dtype=np.float32)
    gtbm[:, :HID] = ln_t_g[None, :]
    gtbm[:, HID:] = ln_t_b[None, :]

    base = {
        "wht": wht.astype(BF16),
        "t1": t1m.astype(BF16),
        "t2": t2m.astype(BF16),
        "w1": w1m.astype(BF16),
        "w2": w2m.astype(BF16),
        "owt": owtm.astype(np.float32),
        "idb": np.eye(SLOT, dtype=np.float32).astype(BF16),
        "idf": np.eye(HID, dtype=np.float32).astype(BF16),
        "b2": ch2_b[:, None].astype(np.float32),
        "ob": obm.astype(np.float32),
        "t1b": t1bv,
        "pk": pkm,
        "gtb": gtbm,
    }

    in_maps = []
    for c in range(NCORES):
        d = dense[c].reshape(NG, 4, 4, K, 106)       # [g, t, u, k, c]
        # pre-transposed: [g, feature, t, u*k] so the head matmul's lhsT
        # (GT) comes straight from DMA with no PE transpose
        d = np.ascontiguousarray(d.transpose(0, 4, 1, 2, 3))  # [g, c, t, u, k]
        d = d.reshape(NG, 106, 4 * SLOT)
        m = dict(base)
        m["xin"] = d
        in_maps.append(m)
    return in_maps, NG, NPC, NPCP, lnt_identity, N


def kernel(**inputs):
    global LAST_RESULT
    from concourse.bass_utils import run_bass_kernel_spmd

    in_maps, NG, NPC, NPCP, lnt_identity, N = _host_prepare(inputs)

    key = (NG, lnt_identity)
    if key not in _CACHE:
        _CACHE[key] = _build_nc(NG, lnt_identity)
    nc = _CACHE[key]

    res = run_bass_kernel_spmd(nc, in_maps, core_ids=list(range(NCORES)))
    LAST_RESULT = res

    parts = []
    remaining = N
    for c in range(NCORES):
        take = min(NPC, remaining)
        parts.append(res.results[c]["y2"].T[:take])
        remaining -= take
    out = np.ascontiguousarray(np.concatenate(parts, axis=0)).astype(np.float32)
    return out


# revision 40
# speedup vs baseline: 1.0406x; 1.0248x over previous
"""Trainium2 Bass kernel for nn_LinkEncoding (gnn_message_passing).

Takes FULL inputs (as produced by reference.setup_inputs()), shards
data-parallel over nodes across 8 NeuronCores, runs a Bass/Tile kernel,
returns the FULL [N, OUT_CH] float32 output.

v3 design notes (vs v2, 3.42ms baseline):
  - All three LayerNorms' per-(slot,t) statistics for the iteration's
    in-flight groups {t: g, c: g-2, h: g-4} land in ONE [120,3,4,6]
    strip; a single batched chain (even/odd bn_stats recombine + magic
    rsqrt + 1 Newton step, 9 DVE ops on [120,12]) replaces 12 bn_aggr +
    3 separate 5-op chains per group.
  - LN_t normalize is 4 per-t tensor_scalar ops (x*ri+nmb in one pass,
    per-partition scalar operands).
  - ch1 bias b1 rides an extra ones-column in zcS (col 101) through the
    transpose into w1 row 101; the 4 per-j Gelus merge into 2 double-
    width Gelus (no per-j bias needed anymore).
  - ch2 bias b2 rides a warm ones-row in hcS (row 100) through the
    back-transpose via idf row 100; the DVE bias-add becomes an ACT
    copy.
  - TM16->acc copy moved DVE->ACT.
  - PSUM: Bx x4, Btp/TM16 1 bank, zcT/Bc/P2f 1 bank, Bh [100,2,512] 2
    banks = 8.
"""

import math
import os
import sys

for _p in ("/opt/trn_rl_repo", "/root/.axon_site/_ro/trn_rl_repo"):
    if os.path.isdir(_p) and _p not in sys.path:
        sys.path.append(_p)

import numpy as np
import ml_dtypes

BF16 = ml_dtypes.bfloat16
FP8 = ml_dtypes.float8_e4m3

# Problem constants (hardcoded per harness contract).
N_NODES = 50000
E_EDGES = 800000
K = 30
HID = 100
TCH = 100
OUT_CH = 100
NCORES = 8

SLOT = 4 * K          # 120 slots (4 nodes) per sub-tile
GW = 16               # nodes per group (4 sub-tiles)
CH_G = 32             # groups per chunk (TM [101, 512] f32 = 1 bank)

MAGIC = 0x5F3759DF

_CACHE = {}
LAST_RESULT = None


def _bcast(ap, n):
    """Broadcast the last (size-1) free dim of `ap` to size n via stride 0."""
    import concourse.bass as bass
    a = list(ap.ap)
    assert a[-1][1] == 1, a
    a[-1] = [0, n]
    return bass.AP(tensor=ap.tensor, offset=ap.offset, ap=a)


def _bcast_mid(ap, n):
    """Insert a stride-0 dim of size n after the partition dim."""
    import concourse.bass as bass
    a = list(ap.ap)
    a.insert(1, [0, n])
    return bass.AP(tensor=ap.tensor, offset=ap.offset, ap=a)


def _build_nc(NG, lnt_identity):
    import concourse.bass as bass
    import concourse.tile as tile
    from concourse import bacc, mybir
    from contextlib import ExitStack

    f32 = mybir.dt.float32
    bf16 = mybir.dt.bfloat16
    i32 = mybir.dt.int32
    AF = mybir.ActivationFunctionType
    OP = mybir.AluOpType

    nc = bacc.Bacc(None, target_bir_lowering=False)

    xin = nc.dram_tensor("xin", [NG, 106, 4 * SLOT], bf16, kind="ExternalInput")
    wht = nc.dram_tensor("wht", [106, HID], bf16, kind="ExternalInput")
    t1 = nc.dram_tensor("t1", [SLOT, 64], bf16, kind="ExternalInput")
    t2 = nc.dram_tensor("t2", [64, SLOT], bf16, kind="ExternalInput")
    fp8 = mybir.dt.float8e4
    w1 = nc.dram_tensor("w1", [102, 4, HID], bf16, kind="ExternalInput")
    w2 = nc.dram_tensor("w2", [HID, 4, HID], bf16, kind="ExternalInput")
    owt = nc.dram_tensor("owt", [101, HID], f32, kind="ExternalInput")
    idb = nc.dram_tensor("idb", [SLOT, SLOT], bf16, kind="ExternalInput")
    idf = nc.dram_tensor("idf", [HID, HID], bf16, kind="ExternalInput")
    b2 = nc.dram_tensor("b2", [HID, 1], f32, kind="ExternalInput")
    ob = nc.dram_tensor("ob", [HID, 1], f32, kind="ExternalInput")
    t1b = nc.dram_tensor("t1b", [64, 1], f32, kind="ExternalInput")
    pk = nc.dram_tensor("pk", [SLOT, 4], f32, kind="ExternalInput")
    gtb = nc.dram_tensor("gtb", [SLOT, 2 * HID], f32, kind="ExternalInput")
    y2 = nc.dram_tensor("y2", [HID, NG * GW], f32, kind="ExternalOutput")

    with tile.TileContext(nc) as tc, ExitStack() as ctx:
        singles = ctx.enter_context(tc.tile_pool(name="singles", bufs=1))
        pgt = ctx.enter_context(tc.tile_pool(name="pgt", bufs=6))
        px = ctx.enter_context(tc.tile_pool(name="px", bufs=3))
        pgel = ctx.enter_context(tc.tile_pool(name="pgel", bufs=3))
        pzt = ctx.enter_context(tc.tile_pool(name="pzt", bufs=3))
        pgh = ctx.enter_context(tc.tile_pool(name="pgh", bufs=2))
        phc = ctx.enter_context(tc.tile_pool(name="phc", bufs=2))
        phts = ctx.enter_context(tc.tile_pool(name="phts", bufs=4))
        pstat = ctx.enter_context(tc.tile_pool(name="pstat", bufs=2))
        ptm = ctx.enter_context(tc.tile_pool(name="ptm", bufs=2))
        # PSUM pools: 3 (Bx) + 2 (btm, zcb) + 2 (bh) + 1 (btT) = 8 banks
        pbx = ctx.enter_context(tc.tile_pool(name="pbx", bufs=3, space="PSUM"))
        pps = ctx.enter_context(tc.tile_pool(name="pps", bufs=1, space="PSUM"))
        pbh = ctx.enter_context(tc.tile_pool(name="pbh", bufs=1, space="PSUM"))
        pbt = ctx.enter_context(tc.tile_pool(name="pbt", bufs=1, space="PSUM"))

        # --- constants -------------------------------------------------
        s_wht = singles.tile([106, HID], bf16)
        nc.sync.dma_start(s_wht[:], wht[:, :])
        s_t1 = singles.tile([SLOT, 64], bf16)
        nc.sync.dma_start(s_t1[:], t1[:, :])
        s_t2 = singles.tile([64, SLOT], bf16)
        nc.sync.dma_start(s_t2[:], t2[:, :])
        s_w1 = singles.tile([102, 4, HID], bf16)
        nc.sync.dma_start(s_w1[:], w1[:, :, :])
        s_w2 = singles.tile([HID, 4, HID], bf16)
        nc.sync.dma_start(s_w2[:], w2[:, :, :])
        s_owt = singles.tile([101, HID], f32)
        nc.sync.dma_start(s_owt[:], owt[:, :])
        s_idb = singles.tile([SLOT, SLOT], bf16)
        nc.sync.dma_start(s_idb[:], idb[:, :])
        s_idf = singles.tile([HID, HID], bf16)
        nc.sync.dma_start(s_idf[:], idf[:, :])
        s_b2 = singles.tile([HID, 1], f32)
        nc.sync.dma_start(s_b2[:], b2[:, :])
        s_ob = singles.tile([HID, 1], f32)
        nc.sync.dma_start(s_ob[:], ob[:, :])
        s_t1b = singles.tile([64, 1], f32)
        nc.sync.dma_start(s_t1b[:], t1b[:, :])
        s_pk = singles.tile([SLOT, 4], f32)
        nc.sync.dma_start(s_pk[:], pk[:, :])
        s_gtb = singles.tile([SLOT, 2 * HID], f32)
        nc.sync.dma_start(s_gtb[:], gtb[:, :])

        nchunks = (NG + CH_G - 1) // CH_G
        PF = 3                     # DMA prefetch depth (groups)
        gts = {}                   # g -> GTs tile
        state = {}                 # g -> per-group tiles live across stages
        accs = {}                  # ci -> SBUF TM accumulator

        def issue_dma(g):
            GTs = pgt.tile([106, 4 * SLOT], bf16, tag="gt", name=f"gts{g}")
            nc.sync.dma_start(GTs[:], xin[g, :, :])
            gts[g] = GTs

        def P1(g):
            GTs = gts.pop(g)
            Bx = pbx.tile([SLOT, 4, HID], f32, tag="bx", name=f"bx{g}")
            for t in range(4):
                nc.tensor.matmul(Bx[:, t, :],
                                 GTs[:, t * SLOT:(t + 1) * SLOT],
                                 s_wht[:], start=(t == 0), stop=True,
                                 skip_group_check=True)
            state[g] = {"Bx": Bx}

        def P2b(g):
            Bx = state[g]["Bx"]
            gel = state[g].pop("gel")
            # tok2 (merged): h_token = x + t2.T @ gel   (accumulate)
            nc.tensor.matmul(Bx[:, :, :], s_t2[:], gel[:, :, :],
                             start=False, stop=True, skip_group_check=True)

        def strip(i):
            """Per-iteration stats strip: slot 0=LN_t(i), 1=LN_c(i-2),
            2=LN_h(i-5)."""
            st = pstat.tile([SLOT, 3, 4, 6], f32, tag="st", name=f"st{i}")
            if not (5 <= i < NG):
                nc.vector.memset(st[:], 0.0)
            return st

        def stats_into(st, sl, src_fn):
            for t in range(4):
                nc.vector.bn_stats(st[:, sl, t, :], src_fn(t))

        def chain(i, st):
            """Batched even/odd recombine + magic rsqrt for all 3 LN slots
            (DVE, end of iteration; consumed next iteration).

            st fields per (ln,t): [50, m_e, cv_e, 50, m_o, cv_o].
            ve = cv_e + cv_o + 25*(m_e-m_o)^2 + 1e-3 = 100*(var + 1e-5);
            ri = 10/sqrt(ve) = 1/sqrt(var+eps) (x10 folded into the last
            Newton step); mm = m_e + m_o = 2*mean; nmb = -mean*ri_t.
            """
            v = nc.vector
            mk = lambda tg: pstat.tile([SLOT, 3, 4], f32, tag=tg,
                                       name=f"{tg}{i}")
            d = mk("chd")
            v.tensor_tensor(d[:], st[:, :, :, 1], st[:, :, :, 4],
                            op=OP.subtract)
            s = mk("chs")
            v.scalar_tensor_tensor(s[:], st[:, :, :, 2], 1e-3,
                                   st[:, :, :, 5], op0=OP.add, op1=OP.add)
            d2 = mk("chd2")
            v.tensor_tensor(d2[:], d[:], d[:], op=OP.mult)
            ve = mk("chve")
            v.scalar_tensor_tensor(ve[:], d2[:], 25.0, s[:], op0=OP.mult,
                                   op1=OP.add)
            y0 = mk("chy0")
            v.tensor_scalar(y0[:].bitcast(i32), ve[:].bitcast(i32), -0.5,
                            float(MAGIC), op0=OP.mult, op1=OP.add)
            t1n = mk("cht1")
            v.tensor_tensor(t1n[:], y0[:], y0[:], op=OP.mult)
            t2n = mk("cht2")
            v.scalar_tensor_tensor(t2n[:], t1n[:], -5.0, ve[:], op0=OP.mult,
                                   op1=OP.mult)
            ri = mk("chri")
            v.scalar_tensor_tensor(ri[:], t2n[:], 15.0, y0[:], op0=OP.add,
                                   op1=OP.mult)
            mm = mk("chmm")
            v.tensor_tensor(mm[:], st[:, :, :, 1], st[:, :, :, 4], op=OP.add)
            nmb = pstat.tile([SLOT, 4], f32, tag="chnm", name=f"chnm{i}")
            v.scalar_tensor_tensor(nmb[:], mm[:, 0, :], -0.5, ri[:, 0, :],
                                   op0=OP.mult, op1=OP.mult)
            return ri, mm, nmb

        def HtS(g):
            """h_token -> bf16 SBUF; frees Bx's PSUM bank (ACT)."""
            Bx = state[g].pop("Bx")
            htS = phts.tile([SLOT, 4, HID], bf16, tag="hts", name=f"hts{g}")
            nc.scalar.activation(htS[:], Bx[:, :, :], AF.Copy)
            state[g]["htS"] = htS

        def Xc(g):
            """x -> bf16 SBUF copy (ACT); feeds stats_t + V1n 4x-mode."""
            Bx = state[g]["Bx"]
            xc = px.tile([SLOT, 4, HID], bf16, tag="xc", name=f"xc{g}")
            nc.scalar.activation(xc[:], Bx[:, :, :], AF.Copy)
            state[g]["xc"] = xc

        def V1n(g, riP, nmbP):
            """LN_t normalize: xS = (x - m)*ri, bf16 SBUF (DVE 4x mode)."""
            xc = state[g].pop("xc")
            xS = px.tile([SLOT, 4, HID], bf16, tag="xs", name=f"xs{g}")
            for t in range(4):
                nc.vector.tensor_scalar(xS[:, t, :], xc[:, t, :],
                                        riP[:, 0, t:t + 1], nmbP[:, t:t + 1],
                                        op0=OP.mult, op1=OP.add)
            if not lnt_identity:
                gt_ = _bcast_mid(s_gtb[:, 0:100], 4)
                bt_ = _bcast_mid(s_gtb[:, 100:200], 4)
                tmp = px.tile([SLOT, 4, HID], f32, tag="lngt")
                nc.vector.scalar_tensor_tensor(
                    tmp[:], xS[:, :, :], 1.0, gt_, op0=OP.mult, op1=OP.mult)
                nc.vector.tensor_tensor(xS[:, :, :], tmp[:], bt_, op=OP.add)
            state[g]["xS"] = xS

        def V2n(g, riP, mmP):
            """LN_c normalize into zcS from bf16 h_token (DVE 4x mode).

            Col 100 = -m*ric, col 101 = 1 (b1 row), cols 102:128 pad
            (memset 1.0, matched by zero rows of w1)."""
            htS = state[g]["htS"]
            zcS = px.tile([SLOT, 4, 102], bf16, tag="zc", name=f"zc{g}")
            nc.vector.memset(zcS[:, :, 101], 1.0)
            nc.vector.scalar_tensor_tensor(zcS[:, :, 100], mmP[:, 1, :],
                                           -0.5, riP[:, 1, :], op0=OP.mult,
                                           op1=OP.mult)
            for t in range(4):
                nc.vector.tensor_scalar(zcS[:, t, 0:100], htS[:, t, :],
                                        riP[:, 1, t:t + 1], None,
                                        op0=OP.mult)
            state[g]["zcS"] = zcS

        def V3bR(g, riP, mmP):
            """LN_h: hhS aug col = -mean; R = pk (x) ri_h (DVE)."""
            hhS = state[g]["hhS"]
            nc.vector.tensor_scalar(hhS[:, :, 100], mmP[:, 2, :], -0.5,
                                    None, op0=OP.mult)
            R = pstat.tile([SLOT, 4, 4], bf16, tag="R", name=f"R{g}")
            nc.vector.tensor_tensor(R[:], _bcast_mid(s_pk[:, :], 4),
                                    _bcast(riP[:, 2, :, None], 4),
                                    op=OP.mult)
            state[g]["R"] = R

        def P2a(g):
            xS = state[g].pop("xS")
            Btp = pps.tile([64, 4, HID], f32, tag="btm", bufs=1,
                           name=f"btp{g}")
            nc.tensor.matmul(Btp[:, :, :], s_t1[:], xS[:, :, :],
                             start=True, stop=True, skip_group_check=True)
            state[g]["Btp"] = Btp

        def P2a2(g):
            Btp = state[g].pop("Btp")
            gel = pgel.tile([64, 4, HID], bf16, tag="gel", name=f"gel{g}")
            nc.scalar.activation(gel[:], Btp[:, :, :], AF.Gelu,
                                 bias=s_t1b[:, 0:1], scale=1.0)
            state[g]["gel"] = gel

        def P3a(g):
            zcS = state[g].pop("zcS")
            # transpose all 128 (padded) channels; the DoubleRow i=1 group
            # is a broadcast duplicate matched by zero rows in w1
            zcT = pps.tile([102, 4, SLOT], bf16, tag="zcb", bufs=1,
                           name=f"zcT{g}")
            for t in range(4):
                nc.tensor.matmul(zcT[:, t, :], zcS[:, t, 0:102], s_idb[:],
                                 is_transpose=True, start=(t == 0),
                                 stop=True, skip_group_check=True)
            zs = pzt.tile([102, 4, SLOT], bf16, tag="zs", name=f"zs{g}")
            nc.scalar.activation(zs[:], zcT[:], AF.Copy)
            state[g]["zs"] = zs

        def P3b(g, half):
            """ch1 pair (j=2*half, 2*half+1), fp8 DoubleRow + merged gelu."""
            zs = state[g]["zs"]
            Bh = pbh.tile([HID, 2, 512], f32, tag="bh", name=f"bh{g}_{half}")
            for jj in range(2):
                j = 2 * half + jj
                nc.tensor.matmul(Bh[:, jj, 0:480], s_w1[:, j, :],
                                 zs[:, :, :], start=True, stop=True)
            gh = state[g].get("gh")
            if gh is None:
                gh = pgh.tile([HID, 4, 4 * SLOT], bf16, tag="gh",
                              name=f"gh{g}")
                state[g]["gh"] = gh
            nc.scalar.activation(gh[:, 2 * half:2 * half + 2, :],
                                 Bh[:, :, 0:480], AF.Gelu)

        def P3c(g):
            """ch2 (fp8 DoubleRow over j-block pairs) + back-transpose."""
            gh = state[g].pop("gh")
            state[g].pop("zs")
            Bc = pbt.tile([HID, 4 * SLOT], f32, tag="btt", bufs=1,
                          name=f"bc{g}")
            for j in range(4):
                nc.tensor.matmul(Bc[:], s_w2[:, j, :], gh[:, j, :],
                                 start=(j == 0), stop=(j == 3))
            hcS = phc.tile([HID, 4 * SLOT], bf16, tag="hc", name=f"hc{g}")
            nc.scalar.activation(hcS[:], Bc[:], AF.Identity,
                                 bias=s_b2[:, 0:1], scale=1.0)
            btT = pbt.tile([SLOT, 4, HID], bf16, tag="btt", name=f"btt{g}")
            for t in range(4):
                nc.tensor.matmul(btT[:, t, :],
                                 hcS[:, t * SLOT:(t + 1) * SLOT],
                                 s_idf[:], is_transpose=True,
                                 start=(t == 0), stop=True,
                                 skip_group_check=True)
            state[g]["btT"] = btT

        def V3a(g):
            """h_channel = h_token + ch2^T (DVE bf16 add)."""
            htS = state[g].pop("htS")
            btT = state[g].pop("btT")
            hhS = px.tile([SLOT, 4, 102], bf16, tag="hh", name=f"hh{g}")
            nc.vector.tensor_tensor(hhS[:, :, 0:100], htS[:, :, :],
                                    btT[:, :, :], op=OP.add)
            state[g]["hhS"] = hhS

        def P4(g):
            st_ = state.pop(g)
            hhS, R = st_["hhS"], st_["R"]
            ci, gi = g // CH_G, g % CH_G
            if gi == 0:
                accs[ci] = ptm.tile([101, GW * CH_G], f32, tag="acc",
                                    name=f"acc{ci}")
            acc = accs[ci]
            TM16 = pps.tile([101, 16], f32, tag="btm", bufs=1,
                            name=f"tm{g}")
            for t in range(4):
                nc.tensor.matmul(TM16[:, 4 * t:4 * t + 4],
                                 hhS[:, t, 0:101], R[:, t, :],
                                 start=(t == 0), stop=True,
                                 skip_group_check=True)
            nc.vector.tensor_copy(acc[:, 16 * gi:16 * gi + 16], TM16[:])
            if gi == CH_G - 1 or g == NG - 1:
                finale(ci)

        def finale(ci):
            acc = accs.pop(ci)
            g0 = ci * CH_G
            gn = min(CH_G, NG - g0)
            nn = GW * gn
            P2f = pps.tile([HID, GW * CH_G], f32, tag="zcb", bufs=1,
                           name=f"p2f{ci}")
            nc.tensor.matmul(P2f[:, :nn], s_owt[:], acc[:, :nn],
                             start=True, stop=True)
            pj = ptm.tile([HID, GW * CH_G], f32, tag="pj", name=f"pj{ci}")
            nc.scalar.activation(pj[:, :nn], P2f[:, :nn], AF.Identity,
                                 bias=s_ob[:, 0:1], scale=1.0)
            nc.sync.dma_start(y2[:, g0 * GW:g0 * GW + nn], pj[:, :nn])

        # --- software-pipelined emission -------------------------------
        # chain(i) covers {LN_t: i, LN_c: i-2, LN_h: i-5}; its outputs are
        # consumed the NEXT iteration (riP/mmP/nmbP) so no engine waits on
        # the in-iteration DVE chain.
        riP = mmP = nmbP = None
        for g in range(min(PF, NG)):
            issue_dma(g)
        for i in range(NG + 6):
            if 0 <= i - 2 < NG:
                P2b(i - 2)
            if i < NG:
                P1(i)
                if i + PF < NG:
                    issue_dma(i + PF)
            if 0 <= i - 1 < NG:
                V1n(i - 1, riP, nmbP)
            if 0 <= i - 6 < NG:
                V3bR(i - 6, riP, mmP)
                P4(i - 6)
            if 0 <= i - 1 < NG:
                P2a(i - 1)
                P2a2(i - 1)
            if i < NG:
                Xc(i)
            if 0 <= i - 4 < NG:
                P3b(i - 4, 0)
            st = strip(i)
            if i < NG:
                xc_ = state[i]["xc"]
                stats_into(st, 0, lambda t: xc_[:, t, :])
            if 0 <= i - 4 < NG:
                P3b(i - 4, 1)
            if 0 <= i - 2 < NG:
                HtS(i - 2)
            if 0 <= i - 3 < NG:
                V2n(i - 3, riP, mmP)
                P3a(i - 3)
            if 0 <= i - 2 < NG:
                ht_ = state[i - 2]["htS"]
                stats_into(st, 1, lambda t: ht_[:, t, :])
            if 0 <= i - 4 < NG:
                P3c(i - 4)
                V3a(i - 4)
            if 0 <= i - 5 < NG:
                hh_ = state[i - 5]["hhS"]
                stats_into(st, 2, lambda t: hh_[:, t, 0:100])
            riP, mmP, nmbP = chain(i, st)
    nc.compile()
    return nc


def _host_prepare(inputs):
    """Build per-core device input maps from the full problem inputs."""
    ea = np.asarray(inputs["edge_attr"], dtype=np.float32)
    et = np.asarray(inputs["edge_time"], dtype=np.float32)
    nb = np.asarray(inputs["node_batch"]).astype(np.int64)
    N = int(np.asarray(inputs["num_nodes"]))
    E = nb.shape[0]

    head_w = np.asarray(inputs["head_w"], dtype=np.float64)
    head_b = np.asarray(inputs["head_b"], dtype=np.float64)
    ln_t_g = np.asarray(inputs["ln_t_g"], dtype=np.float64)
    ln_t_b = np.asarray(inputs["ln_t_b"], dtype=np.float64)
    tok1_w = np.asarray(inputs["tok1_w"], dtype=np.float64)
    tok1_b = np.asarray(inputs["tok1_b"], dtype=np.float64)
    tok2_w = np.asarray(inputs["tok2_w"], dtype=np.float64)
    tok2_b = np.asarray(inputs["tok2_b"], dtype=np.float64)
    ln_c_g = np.asarray(inputs["ln_c_g"], dtype=np.float64)
    ln_c_b = np.asarray(inputs["ln_c_b"], dtype=np.float64)
    ch1_w = np.asarray(inputs["ch1_w"], dtype=np.float64)
    ch1_b = np.asarray(inputs["ch1_b"], dtype=np.float64)
    ch2_w = np.asarray(inputs["ch2_w"], dtype=np.float64)
    ch2_b = np.asarray(inputs["ch2_b"], dtype=np.float64)
    ln_h_g = np.asarray(inputs["ln_h_g"], dtype=np.float64)
    ln_h_b = np.asarray(inputs["ln_h_b"], dtype=np.float64)
    out_w = np.asarray(inputs["out_w"], dtype=np.float64)
    out_b = np.asarray(inputs["out_b"], dtype=np.float64)

    NPC = (N + NCORES - 1) // NCORES          # nodes per core
    NPCP = ((NPC + GW - 1) // GW) * GW        # padded to group multiple
    NG = NPCP // GW

    # --- edge -> slot assignment (stable sort, first K per node) ---
    order = np.argsort(nb, kind="stable")
    snb = nb[order]
    pos = np.arange(E, dtype=np.int64) - np.searchsorted(snb, snb, side="left")
    keep = pos < K
    le = order[keep]                 # edge ids, slot-ordered
    lnode = snb[keep]
    lk = pos[keep]
    core = (lnode // NPC).astype(np.int64)
    nl = (lnode % NPC).astype(np.int64)

    # --- dense slot table [cores, NPCP, K, 106] bf16 ---
    dense = np.zeros((NCORES, NPCP, K, 106), dtype=BF16)
    t64 = et[le].astype(np.float64)
    t2 = t64 * t64
    tp = np.stack([t2, t2 ** 2, t2 ** 3, t2 ** 4, t2 ** 5], axis=1)
    dense[core, nl, lk, 0:5] = tp.astype(np.float32)
    dense[core, nl, lk, 5:105] = ea[le]
    dense[core, nl, lk, 105] = np.float32(1.0)

    # --- folded weights ---
    sqrt_d = math.sqrt(TCH)
    tw = 1.0 / sqrt_d ** np.linspace(0.0, sqrt_d, TCH)  # float64
    W_time = head_w[:, :TCH]
    W_attr = head_w[:, TCH:]
    C = []
    for m in range(6):
        coef = ((-1.0) ** m) / math.factorial(2 * m)
        C.append(coef * (W_time @ (tw ** (2 * m))))     # [HID]
    wht = np.zeros((106, HID), dtype=np.float32)
    for m in range(1, 6):
        wht[m - 1, :] = C[m]
    wht[5:105, :] = W_attr.T
    wht[105, :] = head_b + C[0]

    lnt_identity = bool(np.allclose(ln_t_g, 1.0) and np.allclose(ln_t_b, 0.0))

    t1m = np.zeros((SLOT, 64), dtype=np.float32)
    t2m = np.zeros((64, SLOT), dtype=np.float32)
    for b in range(4):
        t1m[30 * b:30 * b + 30, 16 * b:16 * b + 15] = tok1_w.T
        t2m[16 * b:16 * b + 15, 30 * b:30 * b + 30] = tok2_w.T
    t1bv = np.zeros((64, 1), dtype=np.float32)
    for b in range(4):
        t1bv[16 * b:16 * b + 15, 0] = tok1_b
    # tok2_b dropped: constant per-slot shift is invariant under LN_c /
    # LN_h (which are the only consumers of h_token / h_channel).

    Wg1 = ch1_w * ln_c_g[None, :]
    b1p = ch1_b + ch1_w @ ln_c_b
    w1m = np.zeros((102, 4, HID), dtype=np.float32)
    for j in range(4):
        blk = Wg1[HID * j:HID * (j + 1), :].T          # [100c, 100h]
        w1m[0:100, j, :] = blk
        w1m[100, j, :] = blk.sum(axis=0)               # w1 row-sums
        w1m[101, j, :] = b1p[HID * j:HID * (j + 1)]    # ch1 bias (ones col)
    w2m = np.stack([ch2_w[:, HID * j:HID * (j + 1)].T for j in range(4)],
                   axis=1)                              # [100h, 4, 100c]

    OWg = out_w * ln_h_g[None, :]
    owtm = np.zeros((101, HID), dtype=np.float32)
    owtm[0:100, :] = OWg.T
    owtm[100, :] = OWg.sum(axis=1)
    obm = (out_b + out_w @ ln_h_b)[:, None]

    pkm = np.zeros((SLOT, 4), dtype=np.float32)
    for b in range(4):
        pkm[30 * b:30 * b + 30, b] = 1.0 / K

    gtbm = np.zeros((SLOT, 2 * HID), dtype=np.float32)
    gtbm[:, :HID] = ln_t_g[None, :]
    gtbm[:, HID:] = ln_t_b[None, :]

    base = {
        "wht": wht.astype(BF16),
        "t1": t1m.astype(BF16),
        "t2": t2m.astype(BF16),
        "w1": w1m.astype(BF16),
        "w2": w2m.astype(BF16),
        "owt": owtm.astype(np.float32),
        "idb": np.eye(SLOT, dtype=np.float32).astype(BF16),
        "idf": np.eye(HID, dtype=np.float32).astype(BF16),
        "b2": ch2_b[:, None].astype(np.float32),
        "ob": obm.astype(np.float32),
        "t1b": t1bv,
        "pk": pkm,
        "gtb": gtbm,
    }

    in_maps = []
    for c in range(NCORES):
        d = dense[c].reshape(NG, 4, 4, K, 106)       # [g, t, u, k, c]
        # pre-transposed: [g, feature, t, u*k] so the head matmul's lhsT
        # (GT) comes straight from DMA with no PE transpose
        d = np.ascontiguousarray(d.transpose(0, 4, 1, 2, 3))  # [g, c, t, u, k]
        d = d.reshape(NG, 106, 4 * SLOT)
        m = dict(base)
        m["xin"] = d
        in_maps.append(m)
    return in_maps, NG, NPC, NPCP, lnt_identity, N


def kernel(**inputs):
    global LAST_RESULT
    from concourse.bass_utils import run_bass_kernel_spmd

    in_maps, NG, NPC, NPCP, lnt_identity, N = _host_prepare(inputs)

    key = (NG, lnt_identity)
    if key not in _CACHE:
        _CACHE[key] = _build_nc(NG, lnt_identity)
    nc = _CACHE[key]

    res = run_bass_kernel_spmd(nc, in_maps, core_ids=list(range(NCORES)))
    LAST_RESULT = res

    parts = []
    remaining = N
    for c in range(NCORES):
        take = min(NPC, remaining)
        parts.append(res.results[c]["y2"].T[:take])
        remaining -= take
    out = np.ascontiguousarray(np.concatenate(parts, axis=0)).astype(np.float32)
    return out
